# revision 25
# baseline (speedup 1.0000x reference)
"""NamedSlotMemory Trainium2 Bass kernel.

Contract: kernel(**inputs) takes FULL (unsharded) inputs, shards batch B=32
across 8 NeuronCores (4 batches each), runs one SPMD Bass program, gathers and
returns (augmented, slots) as numpy float32.

Per-core plan (4 local batches, T=4096, DI=D=256, N=16 slots, 3 GRU iters):
  phase A: per 128-row x tile: LN stats via bn_stats; x_std=(x-m)*r -> bf16
           (kept resident, doubles as the "v source"); stream x to aug[:, :256].
  phase B: transpose x_std per 512-chunk on PE; kT[d,t] = (Wk*g_ln).T matmuls
           (bf16, LN bias folded to a per-partition ACT bias on the copy).
  iters:   logits[t,n] = kT-slices (PE weights) x qT (bf16); softmax over the
           free dim with fused Exp+row-sum; attn bf16; colsum via ones-matmul;
           updates = (attnT @ x_std) @ (Wv*g_ln).T with 1/(colsum+eps) folded
           into the PSUM->SBUF copy; GRU + LN + MLP batched over all local
           batches on [64,256] tiles in fp32/fp32r.
  tail:    soft_slot = attnT @ slots per t-tile -> aug[:, 256:]; slots out.
"""

from contextlib import ExitStack

import numpy as np

import concourse.bass as bass
import concourse.bacc as bacc
import concourse.tile as tile
from concourse import mybir
from concourse.bass import ds, ts
from concourse.bass_utils import run_bass_kernel_spmd
from concourse.masks import make_identity

F32 = mybir.dt.float32
F32R = mybir.dt.float32r
BF16 = mybir.dt.bfloat16
AF = mybir.ActivationFunctionType
OP = mybir.AluOpType

B, T, DI, D, N, NITERS, MLP = 32, 4096, 256, 256, 16, 3, 512
NCORES = 8
BL = B // NCORES
EPS_LN = 1e-5
EPS_RENORM = 1e-8

WEIGHT_NAMES = [
    "ln_in_g", "ln_in_b", "ln_slot_g", "ln_slot_b", "ln_mlp_g", "ln_mlp_b",
    "Wq", "bq", "Wk", "bk", "Wv", "bv", "W_ih", "b_ih", "W_hh", "b_hh",
    "W1", "b1", "W2", "b2", "slot_init",
]


def build_kernel(t_len=T, bl=BL, slot_r=True, dbg=False):
    nc = bacc.Bacc("TRN2", target_bir_lowering=False, debug=False,
                   num_devices=NCORES)
    x_d = nc.dram_tensor("x", [bl, t_len, DI], F32, kind="ExternalInput")
    w = {
        name: nc.dram_tensor(name, shape, F32, kind="ExternalInput")
        for name, shape in [
            ("ln_in_g", [DI]), ("ln_in_b", [DI]),
            ("ln_slot_g", [D]), ("ln_slot_b", [D]),
            ("ln_mlp_g", [D]), ("ln_mlp_b", [D]),
            ("Wq", [D, D]), ("bq", [D]),
            ("Wk", [D, DI]), ("bk", [D]),
            ("Wv", [D, DI]), ("bv", [D]),
            ("W_ih", [3 * D, D]), ("b_ih", [3 * D]),
            ("W_hh", [3 * D, D]), ("b_hh", [3 * D]),
            ("W1", [MLP, D]), ("b1", [MLP]),
            ("W2", [D, MLP]), ("b2", [D]),
            ("slot_init", [1, N, D]),
        ]
    }
    aug_d = nc.dram_tensor("aug", [bl, t_len, DI + D], F32, kind="ExternalOutput")
    slots_d = nc.dram_tensor("slots_out", [bl, N, D], F32, kind="ExternalOutput")

    dbg_d = {}
    if dbg:
        TCd = t_len // 128
        NBd = bl * N
        for name, shape, dt_ in [
            ("dbg_xs", [128, TCd, DI], BF16),
            ("dbg_kT", [128, 2, t_len], BF16),
            ("dbg_attn", [128, TCd, N], BF16),
            ("dbg_q0", [128, 2, 32 * bl], BF16),
            ("dbg_u00", [N, DI], F32),
            ("dbg_upd0", [32 * bl, D], F32),
            ("dbg_slots0", [32 * bl, D], F32),
            ("dbg_slotsg0", [32 * bl, D], F32),
        ]:
            dbg_d[name] = nc.dram_tensor(name, shape, dt_, kind="ExternalOutput")

    with tile.TileContext(nc) as tc:
        with ExitStack() as ctx:
            _emit(ctx, tc, nc, x_d, w, aug_d, slots_d, t_len, bl, slot_r, dbg_d)
    nc.compile()
    return nc


def _emit(ctx, tc, nc, x_d, w, aug_d, slots_d, t_len, bl, slot_r, dbg_d=None):
    P = 128
    TC = t_len // P
    TC8 = t_len // 512
    NB = bl * N          # logical stacked rows
    NP = 32 * bl         # padded: batch b at partition 32*b
    SR = F32R if slot_r else F32
    x_ap = x_d.ap()
    aug_ap = aug_d.ap()

    const = ctx.enter_context(tc.tile_pool(name="const", bufs=1))
    persist = ctx.enter_context(tc.tile_pool(name="persist", bufs=1))
    xtp = ctx.enter_context(tc.tile_pool(name="xtp", bufs=1))
    xin = ctx.enter_context(tc.tile_pool(name="xin", bufs=2))
    stats = ctx.enter_context(tc.tile_pool(name="stats", bufs=4))
    small = ctx.enter_context(tc.tile_pool(name="small", bufs=2))
    one = ctx.enter_context(tc.tile_pool(name="one", bufs=1))
    ssout = ctx.enter_context(tc.tile_pool(name="ssout", bufs=2))
    ps = ctx.enter_context(tc.tile_pool(name="ps", bufs=2, space="PSUM"))

    # ---------------- constants ----------------
    I128f = const.tile([P, P], F32, name="I128f")
    make_identity(nc, I128f)
    I128b = const.tile([P, P], BF16, name="I128b")
    nc.any.tensor_copy(I128b, I128f)

    eps_ln = const.tile([P, 1], F32, name="eps_ln")
    nc.vector.memset(eps_ln, EPS_LN)
    ones_col_b = const.tile([P, 1], BF16, name="ones_col_b")
    nc.vector.memset(ones_col_b, 1.0)
    ones_row_b = const.tile([1, P], BF16, name="ones_row_b")
    nc.vector.memset(ones_row_b, 1.0)

    def col2(dram1d, name):  # [256] -> [128, 2] f32, di = c*128 + p
        t_ = const.tile([P, 2], F32, name=name)
        nc.sync.dma_start(t_, dram1d.ap().rearrange("(c p) -> p c", p=P))
        return t_

    g_in = col2(w["ln_in_g"], "g_in")
    b_in = col2(w["ln_in_b"], "b_in")
    b_sl = col2(w["ln_slot_b"], "b_sl")
    g_sl = col2(w["ln_slot_g"], "g_sl")
    g_ml = col2(w["ln_mlp_g"], "g_ml")
    b_ml = col2(w["ln_mlp_b"], "b_ml")
    bk_c = col2(w["bk"], "bk_c")
    bq_c = col2(w["bq"], "bq_c")

    # bias rows kept in bf16 (only used as K=1 matmul rhs; zero in practice)
    bih_row = const.tile([1, 3 * D], BF16, name="bih_row")
    bhh_row = const.tile([1, 3 * D], BF16, name="bhh_row")
    c2_row = const.tile([1, D], BF16, name="c2_row")

    # -------- weights: load rows, PE-transpose, fold LN gammas --------
    WkgT = const.tile([P, 2, DI], BF16, name="WkgT")    # [di, d] bf16
    WqgT = const.tile([P, 2, D], BF16, name="WqgT")
    WvgT = const.tile([P, 2, D], SR, name="WvgT")      # [di, d] f32 (iter use)
    WihT = const.tile([P, 2, 3 * D], SR, name="WihT")  # [d, gate]
    WhhT = const.tile([P, 2, 3 * D], SR, name="WhhT")
    W1gT = const.tile([P, 2, MLP], SR, name="W1gT")    # [d, m]
    W2T = const.tile([P, 4, D], SR, name="W2T")        # [m, d]
    ck = const.tile([P, 2], F32, name="ck")             # b_ln@Wk.T + bk (col)
    cq16 = const.tile([P, 2], F32, name="cq16")         # (b_ln@Wq.T + bq)/16
    cv_row = const.tile([1, D], BF16, name="cv_row")    # b_ln@Wv.T + bv (row)
    c1_row = const.tile([1, MLP], BF16, name="c1_row")  # b_ln@W1.T + b1

    with tc.tile_pool(name="wsetup", bufs=1) as wsu:
        def load_rows(dram, nrow, ncol, tag="raw"):
            t_ = wsu.tile([P, nrow // P, ncol], F32, tag=tag)
            nc.sync.dma_start(t_, dram.ap().rearrange("(po pi) f -> pi po f", pi=P))
            return t_

        def row_tmp(dram1d, n, tag="brow"):
            t_ = wsu.tile([1, n], F32, tag=tag)
            nc.sync.dma_start(t_, dram1d.ap().unsqueeze(0))
            return t_

        for dst, src, n in [(bih_row, w["b_ih"], 3 * D),
                            (bhh_row, w["b_hh"], 3 * D),
                            (c2_row, w["b2"], D)]:
            nc.any.tensor_copy(dst, row_tmp(src, n))

        def transpose_to(dst, raw, rch, fch):
            for rc in range(rch):
                for fc in range(fch):
                    pt = ps.tile([P, P], F32, tag="ps_t")
                    nc.tensor.transpose(pt, raw[:, rc, ds(fc * P, P)], I128f)
                    nc.any.tensor_copy(dst[:, fc, ds(rc * P, P)], pt)

        def cvec_col(tmpT, bcol, badd, dst, scale=None):
            for dc in range(2):
                pc = ps.tile([P, 1], F32, tag="ps_c")
                for dik in range(2):
                    nc.tensor.matmul(pc, lhsT=tmpT[:, dik, ds(dc * P, P)],
                                     rhs=bcol[:, dik:dik + 1],
                                     start=dik == 0, stop=dik == 1)
                if scale is None:
                    nc.vector.tensor_scalar(dst[:, dc:dc + 1], pc,
                                            badd[:, dc:dc + 1], None, op0=OP.add)
                else:
                    nc.vector.tensor_scalar(dst[:, dc:dc + 1], pc,
                                            badd[:, dc:dc + 1], scale,
                                            op0=OP.add, op1=OP.mult)

        def cvec_row(tmpT, bcol, badd, dst, width):
            pc = ps.tile([1, width], F32, tag="ps_c")
            for dik in range(2):
                nc.tensor.matmul(pc, lhsT=bcol[:, dik:dik + 1],
                                 rhs=tmpT[:, dik, :],
                                 start=dik == 0, stop=dik == 1)
            nc.vector.tensor_tensor(dst, pc, badd, OP.add)

        # Wk -> WkgT bf16 + ck
        raw = load_rows(w["Wk"], D, DI)
        tmpT = wsu.tile([P, 2, D], F32, tag="tmpT")
        transpose_to(tmpT, raw, 2, 2)
        cvec_col(tmpT, b_in, bk_c, ck)
        for dik in range(2):
            nc.vector.tensor_scalar_mul(WkgT[:, dik, :], tmpT[:, dik, :],
                                        g_in[:, dik:dik + 1])
        # Wq -> WqgT bf16 + cq16
        raw = load_rows(w["Wq"], D, D)
        tmpT = wsu.tile([P, 2, D], F32, tag="tmpT")
        transpose_to(tmpT, raw, 2, 2)
        cvec_col(tmpT, b_sl, bq_c, cq16, scale=1.0 / 16.0)
        for dik in range(2):
            nc.vector.tensor_scalar_mul(WqgT[:, dik, :], tmpT[:, dik, :],
                                        g_sl[:, dik:dik + 1])
        # Wv -> WvgT f32 + cv_row
        raw = load_rows(w["Wv"], D, DI)
        tmpT = wsu.tile([P, 2, D], F32, tag="tmpT")
        transpose_to(tmpT, raw, 2, 2)
        cvec_row(tmpT, b_in, row_tmp(w["bv"], D), cv_row, D)
        for dik in range(2):
            nc.vector.tensor_scalar_mul(WvgT[:, dik, :], tmpT[:, dik, :],
                                        g_in[:, dik:dik + 1])
        # W_ih / W_hh plain transposes (halved loads to bound the raw tag)
        for name_, dstT in [("W_ih", WihT), ("W_hh", WhhT)]:
            for half in range(2):
                rawh = wsu.tile([P, 3, D], F32, tag="raw")
                nc.sync.dma_start(
                    rawh, w[name_].ap().rearrange("(po pi) f -> pi po f", pi=P)[
                        :, ds(half * 3, 3), :])
                for rc in range(3):
                    for fc in range(2):
                        pt = ps.tile([P, P], F32, tag="ps_t")
                        nc.tensor.transpose(pt, rawh[:, rc, ds(fc * P, P)], I128f)
                        nc.any.tensor_copy(
                            dstT[:, fc, ds((half * 3 + rc) * P, P)], pt)
        # W1 -> W1gT + c1
        raw = load_rows(w["W1"], MLP, D)
        tmp1 = wsu.tile([P, 2, MLP], F32, tag="tmpT")
        transpose_to(tmp1, raw, 4, 2)
        cvec_row(tmp1, b_ml, row_tmp(w["b1"], MLP), c1_row, MLP)
        for dik in range(2):
            nc.vector.tensor_scalar_mul(W1gT[:, dik, :], tmp1[:, dik, :],
                                        g_ml[:, dik:dik + 1])
        # W2 -> W2T
        transpose_to(W2T, load_rows(w["W2"], D, MLP), 2, 4)

    # ---------------- state ----------------
    slots = persist.tile([NP, D], F32, name="slots")
    nc.vector.memset(slots, 0.0)
    for b in range(bl):
        nc.sync.dma_start(slots[ds(32 * b, N), :], w["slot_init"].ap()[0])

    kT = [persist.tile([P, 2, t_len], BF16, name=f"kT{b}") for b in range(bl)]
    xs = [persist.tile([P, TC, DI], BF16, name=f"xs{b}") for b in range(bl)]
    attn = [persist.tile([P, TC, N], BF16, name=f"attn{b}") for b in range(bl)]

    # ---------------- phase A + B ----------------
    for b in range(bl):
        for tcx in range(TC):
            xt = xin.tile([P, DI], F32, tag="xt")
            nc.sync.dma_start(xt, x_ap[b, ts(tcx, P), :])
            st = stats.tile([P, 6], F32, tag="st")
            nc.vector.bn_stats(st, xt)
            mv = stats.tile([P, 2], F32, tag="mv")
            nc.vector.bn_aggr(mv, st)
            sq = stats.tile([P, 1], F32, tag="sq")
            nc.scalar.activation(sq, mv[:, 1:2], AF.Sqrt, bias=eps_ln)
            rc = stats.tile([P, 1], F32, tag="rc")
            nc.vector.reciprocal(rc, sq)
            nc.vector.tensor_scalar(xs[b][:, tcx, :], xt, mv[:, 0:1], rc,
                                    op0=OP.subtract, op1=OP.mult)
            nc.sync.dma_start(aug_ap[b, ts(tcx, P), 0:DI], xt)

        for tcc in range(TC8):
            xT = xtp.tile([P, 2, 512], BF16, tag="xT")
            for j in range(4):
                tcx = tcc * 4 + j
                for c in range(2):
                    pt = ps.tile([P, P], BF16, tag="ps_t")
                    nc.tensor.transpose(pt, xs[b][:, tcx, ds(c * P, P)], I128b)
                    nc.any.tensor_copy(xT[:, c, ts(j, P)], pt)
            for dc in range(2):
                pk = ps.tile([P, 512], F32, tag="ps_mm")
                for dik in range(2):
                    nc.tensor.matmul(pk, lhsT=WkgT[:, dik, ds(dc * P, P)],
                                     rhs=xT[:, dik, :],
                                     start=dik == 0, stop=dik == 1)
                nc.scalar.activation(kT[b][:, dc, ts(tcc, 512)], pk,
                                     AF.Identity, bias=ck[:, dc:dc + 1])

    if dbg_d:
        nc.sync.dma_start(dbg_d["dbg_xs"].ap(), xs[0][:])
        nc.sync.dma_start(dbg_d["dbg_kT"].ap(), kT[0][:])

    # ---------------- iterations ----------------
    for it in range(NITERS):
        stS = stats.tile([NP, 6], F32, tag="stS")
        nc.vector.bn_stats(stS, slots)
        mvS = stats.tile([NP, 2], F32, tag="mvS")
        nc.vector.bn_aggr(mvS, stS)
        sqS = stats.tile([NP, 1], F32, tag="sqS")
        nc.scalar.activation(sqS, mvS[:, 1:2], AF.Sqrt, bias=eps_ln[:NP])
        rcS = stats.tile([NP, 1], F32, tag="rcS")
        nc.vector.reciprocal(rcS, sqS)
        ln_s = small.tile([NP, D], BF16, tag="ln_s")
        nc.vector.tensor_scalar(ln_s, slots, mvS[:, 0:1], rcS,
                                op0=OP.subtract, op1=OP.mult)
        lsT = small.tile([P, 2, NP], BF16, tag="lsT")
        for c in range(2):
            pt = ps.tile([P, NP], BF16, tag="ps_t")
            nc.tensor.transpose(pt, ln_s[:, ds(c * P, P)], I128b[:NP, :NP])
            nc.any.tensor_copy(lsT[:, c, :], pt)
        qT = small.tile([P, 2, NP], BF16, tag="qT")
        for dc in range(2):
            pq = ps.tile([P, NP], F32, tag="ps_t")
            for dik in range(2):
                nc.tensor.matmul(pq, lhsT=WqgT[:, dik, ds(dc * P, P)],
                                 rhs=lsT[:, dik, :],
                                 start=dik == 0, stop=dik == 1)
            nc.scalar.activation(qT[:, dc, :], pq, AF.Identity,
                                 bias=cq16[:, dc:dc + 1], scale=1.0 / 16.0)

        updates = small.tile([NP, D], F32, tag="updates")
        nc.vector.memset(updates, 0.0)
        for b in range(bl):
            for tcx in range(TC):
                pl = ps.tile([P, N], F32, tag="ps_s")
                for dik in range(2):
                    nc.tensor.matmul(pl, lhsT=kT[b][:, dik, ts(tcx, P)],
                                     rhs=qT[:, dik, ds(32 * b, N)],
                                     start=dik == 0, stop=dik == 1)
                ae = stats.tile([P, N], F32, tag="ae")
                rs = stats.tile([P, 1], F32, tag="rs")
                nc.scalar.activation(ae, pl, AF.Exp, accum_out=rs)
                rr = stats.tile([P, 1], F32, tag="rr")
                nc.vector.reciprocal(rr, rs)
                nc.vector.tensor_scalar_mul(attn[b][:, tcx, :], ae, rr)
            # colsum over t via ones-matmul -> [N, 1]
            pc = ps.tile([N, 1], F32, tag="ps_c")
            for tcx in range(TC):
                nc.tensor.matmul(pc, lhsT=attn[b][:, tcx, :], rhs=ones_col_b,
                                 start=tcx == 0, stop=tcx == TC - 1)
            cs = stats.tile([N, 1], F32, tag="cs")
            nc.vector.tensor_scalar(cs, pc, EPS_RENORM, None, op0=OP.add)
            wsc = stats.tile([N, 1], F32, tag="wsc")
            nc.vector.reciprocal(wsc, cs)
            # U0 = attn.T @ x_std  [N, DI]
            pu = ps.tile([N, DI], F32, tag="ps_s")
            for tcx in range(TC):
                nc.tensor.matmul(pu, lhsT=attn[b][:, tcx, :],
                                 rhs=xs[b][:, tcx, :],
                                 start=tcx == 0, stop=tcx == TC - 1)
            u0 = small.tile([N, DI], F32, tag="u0")
            nc.vector.tensor_scalar_mul(u0, pu, wsc)
            u0T = small.tile([P, 2, N], SR, tag="u0T")
            for c in range(2):
                pt = ps.tile([P, N], F32, tag="ps_t")
                nc.tensor.transpose(pt, u0[:, ds(c * P, P)], I128f[:N, :N])
                nc.any.tensor_copy(u0T[:, c, :], pt)
            # updates_b = (U0 @ WvgT + cv) * wsc
            pub = ps.tile([N, D], F32, tag="ps_s")
            for dik in range(2):
                nc.tensor.matmul(pub, lhsT=u0T[:, dik, :],
                                 rhs=WvgT[:, dik, :],
                                 start=dik == 0, stop=False)
            nc.tensor.matmul(pub, lhsT=ones_row_b[:, :N],
                             rhs=cv_row, start=False, stop=True)
            nc.any.tensor_copy(updates[ds(32 * b, N), :], pub)
            if dbg_d and it == 0 and b == 0:
                nc.sync.dma_start(dbg_d["dbg_u00"].ap(), u0)

        # ---- GRU over stacked batches ----
        updT = small.tile([P, 2, NP], SR, tag="updT")
        slT = small.tile([P, 2, NP], SR, tag="slT")
        for c in range(2):
            pt = ps.tile([P, NP], F32, tag="ps_t")
            nc.tensor.transpose(pt, updates[:, ds(c * P, P)], I128f[:NP, :NP])
            nc.any.tensor_copy(updT[:, c, :], pt)
            pt2 = ps.tile([P, NP], F32, tag="ps_t")
            nc.tensor.transpose(pt2, slots[:, ds(c * P, P)], I128f[:NP, :NP])
            nc.any.tensor_copy(slT[:, c, :], pt2)

        # r,z pre-activations: updT@Wih + slT@Whh + biases, summed in one PSUM
        prz = ps.tile([NP, 512], F32, tag="ps_mm")
        first = True
        for lhsT_t, w_t in [(updT, WihT), (slT, WhhT)]:
            for dik in range(2):
                nc.tensor.matmul(prz, lhsT=lhsT_t[:, dik, :],
                                 rhs=w_t[:, dik, 0:512],
                                 start=first, stop=False)
                first = False
        nc.tensor.matmul(prz, lhsT=ones_row_b[:, :NP], rhs=bih_row[:, 0:512],
                         start=False, stop=False)
        nc.tensor.matmul(prz, lhsT=ones_row_b[:, :NP], rhs=bhh_row[:, 0:512],
                         start=False, stop=True)
        # n-gate halves stay separate (r gates nh only)
        def n_half(lhsT_t, w_t, bias_row, tag):
            pg = ps.tile([NP, D], F32, tag=tag)
            for dik in range(2):
                nc.tensor.matmul(pg, lhsT=lhsT_t[:, dik, :],
                                 rhs=w_t[:, dik, 512:768],
                                 start=dik == 0, stop=False)
            nc.tensor.matmul(pg, lhsT=ones_row_b[:, :NP],
                             rhs=bias_row[:, 512:768], start=False, stop=True)
            return pg

        pnx = n_half(updT, WihT, bih_row, "ps_s")
        pnh = n_half(slT, WhhT, bhh_row, "ps_s")

        def tt(in0, in1, op, tag="gtmp"):
            o = small.tile([NP, D], F32, tag=tag)
            nc.vector.tensor_tensor(o, in0, in1, op)
            return o

        r_g = one.tile([NP, D], F32, tag="r_g")
        nc.scalar.activation(r_g, prz[:, 0:D], AF.Sigmoid)
        z_g = one.tile([NP, D], F32, tag="z_g")
        nc.scalar.activation(z_g, prz[:, D:2 * D], AF.Sigmoid)
        t1 = tt(r_g, pnh, OP.mult)
        t2 = tt(pnx, t1, OP.add)
        n_g = one.tile([NP, D], F32, tag="n_g")
        nc.scalar.activation(n_g, t2, AF.Tanh)
        d1 = tt(slots, n_g, OP.subtract)
        d2 = tt(z_g, d1, OP.mult)
        nc.vector.tensor_tensor(slots, n_g, d2, OP.add)
        if dbg_d and it == 0:
            nc.sync.dma_start(dbg_d["dbg_q0"].ap(), qT[:])
            nc.sync.dma_start(dbg_d["dbg_upd0"].ap(), updates)
            nc.sync.dma_start(dbg_d["dbg_slotsg0"].ap(), slots)

        # ---- MLP residual ----
        stM = stats.tile([NP, 6], F32, tag="stS")
        nc.vector.bn_stats(stM, slots)
        mvM = stats.tile([NP, 2], F32, tag="mvS")
        nc.vector.bn_aggr(mvM, stM)
        sqM = stats.tile([NP, 1], F32, tag="sqS")
        nc.scalar.activation(sqM, mvM[:, 1:2], AF.Sqrt, bias=eps_ln[:NP])
        rcM = stats.tile([NP, 1], F32, tag="rcS")
        nc.vector.reciprocal(rcM, sqM)
        ln_h = one.tile([NP, D], F32, tag="ln_h")
        nc.vector.tensor_scalar(ln_h, slots, mvM[:, 0:1], rcM,
                                op0=OP.subtract, op1=OP.mult)
        lhT = small.tile([P, 2, NP], SR, tag="lhT")
        for c in range(2):
            pt = ps.tile([P, NP], F32, tag="ps_t")
            nc.tensor.transpose(pt, ln_h[:, ds(c * P, P)], I128f[:NP, :NP])
            nc.any.tensor_copy(lhT[:, c, :], pt)
        pm1 = ps.tile([NP, MLP], F32, tag="ps_mm")
        for dik in range(2):
            nc.tensor.matmul(pm1, lhsT=lhT[:, dik, :],
                             rhs=W1gT[:, dik, :],
                             start=dik == 0, stop=False)
        nc.tensor.matmul(pm1, lhsT=ones_row_b[:, :NP],
                         rhs=c1_row, start=False, stop=True)
        h1 = one.tile([NP, MLP], F32, tag="h1")
        nc.scalar.activation(h1, pm1, AF.Relu)
        h1T = one.tile([P, 4, NP], SR, tag="h1T")
        for mc in range(4):
            pt = ps.tile([P, NP], F32, tag="ps_t")
            nc.tensor.transpose(pt, h1[:, ds(mc * P, P)], I128f[:NP, :NP])
            nc.any.tensor_copy(h1T[:, mc, :], pt)
        pm2 = ps.tile([NP, D], F32, tag="ps_s")
        for mc in range(4):
            nc.tensor.matmul(pm2, lhsT=h1T[:, mc, :],
                             rhs=W2T[:, mc, :],
                             start=mc == 0, stop=False)
        nc.tensor.matmul(pm2, lhsT=ones_row_b[:, :NP],
                         rhs=c2_row, start=False, stop=True)
        nc.vector.tensor_tensor(slots, slots, pm2, OP.add)
        if dbg_d and it == 0:
            nc.sync.dma_start(dbg_d["dbg_slots0"].ap(), slots)

    # ---------------- tail ----------------
    for b in range(bl):
        sb16 = small.tile([N, D], BF16, tag="sb16")
        nc.any.tensor_copy(sb16, slots[ds(32 * b, N), :])
        for tcx in range(TC):
            pa = ps.tile([N, P], BF16, tag="ps_c")
            nc.tensor.transpose(pa, attn[b][:, tcx, :], I128b)
            at = small.tile([N, P], BF16, tag="at")
            nc.any.tensor_copy(at, pa)
            pss = ps.tile([P, D], F32, tag="ps_s")
            nc.tensor.matmul(pss, lhsT=at, rhs=sb16, start=True, stop=True)
            so = ssout.tile([P, D], F32, tag="so")
            nc.any.tensor_copy(so, pss)
            nc.sync.dma_start(aug_ap[b, ts(tcx, P), DI:DI + D], so)
    for b in range(bl):
        nc.sync.dma_start(slots_d.ap()[b], slots[ds(32 * b, N), :])
    if dbg_d:
        nc.sync.dma_start(dbg_d["dbg_attn"].ap(), attn[0][:])


_NC_CACHE = {}


def get_nc(t_len=T, bl=BL, slot_r=True):
    key = (t_len, bl, slot_r)
    if key not in _NC_CACHE:
        _NC_CACHE[key] = build_kernel(t_len, bl, slot_r)
    return _NC_CACHE[key]


def kernel(**inputs):
    arrs = {k: np.ascontiguousarray(np.asarray(v, dtype=np.float32))
            for k, v in inputs.items()}
    nc = get_nc()
    x = arrs["x"]
    in_maps = []
    for c in range(NCORES):
        m = {name: arrs[name] for name in WEIGHT_NAMES}
        m["x"] = np.ascontiguousarray(x[c * BL:(c + 1) * BL])
        in_maps.append(m)
    res = run_bass_kernel_spmd(nc, in_maps, core_ids=list(range(NCORES)))
    aug = np.concatenate([r["aug"] for r in res.results], axis=0)
    slots = np.concatenate([r["slots_out"] for r in res.results], axis=0)
    return aug, slots


# revision 41
# speedup vs baseline: 28761.5789x; 28761.5789x over previous
"""NamedSlotMemory Trainium2 Bass kernel.

Contract: kernel(**inputs) takes FULL (unsharded) inputs, shards batch B=32
across 8 NeuronCores (4 batches each), runs one SPMD Bass program, gathers and
returns (augmented, slots) as numpy float32.

Per-core plan (4 local batches, T=4096, DI=D=256, N=16 slots, 3 GRU iters):
  phase A: per 128-row x tile: LN stats via bn_stats; x_std=(x-m)*r -> bf16
           (kept resident, doubles as the "v source"); stream x to aug[:, :256].
  phase B: transpose x_std per 512-chunk on PE; kT[d,t] = (Wk*g_ln).T matmuls
           (bf16, LN bias folded to a per-partition ACT bias on the copy).
  iters:   logits[t,n] = kT-slices (PE weights) x qT (bf16); softmax over the
           free dim with fused Exp+row-sum; attn bf16; colsum via ones-matmul;
           updates = (attnT @ x_std) @ (Wv*g_ln).T with 1/(colsum+eps) folded
           into the PSUM->SBUF copy; GRU + LN + MLP batched over all local
           batches on [64,256] tiles in fp32/fp32r.
  tail:    soft_slot = attnT @ slots per t-tile -> aug[:, 256:]; slots out.
"""

from contextlib import ExitStack

import numpy as np

import concourse.bass as bass
import concourse.bacc as bacc
import concourse.tile as tile
from concourse import mybir
from concourse.bass import ds, ts
from concourse.bass_utils import run_bass_kernel_spmd
from concourse.masks import make_identity

F32 = mybir.dt.float32
F32R = mybir.dt.float32r
BF16 = mybir.dt.bfloat16
AF = mybir.ActivationFunctionType
OP = mybir.AluOpType

B, T, DI, D, N, NITERS, MLP = 32, 4096, 256, 256, 16, 3, 512
NCORES = 8
BL = B // NCORES
EPS_LN = 1e-5
EPS_RENORM = 1e-8

WEIGHT_NAMES = [
    "ln_in_g", "ln_in_b", "ln_slot_g", "ln_slot_b", "ln_mlp_g", "ln_mlp_b",
    "Wq", "bq", "Wk", "bk", "Wv", "bv", "W_ih", "b_ih", "W_hh", "b_hh",
    "W1", "b1", "W2", "b2", "slot_init",
]


def build_kernel(t_len=T, bl=BL, slot_r=True, dbg=False):
    nc = bacc.Bacc("TRN2", target_bir_lowering=False, debug=False,
                   num_devices=NCORES)
    x_d = nc.dram_tensor("x", [bl, t_len, DI], F32, kind="ExternalInput")
    w = {
        name: nc.dram_tensor(name, shape, F32, kind="ExternalInput")
        for name, shape in [
            ("ln_in_g", [DI]), ("ln_in_b", [DI]),
            ("ln_slot_g", [D]), ("ln_slot_b", [D]),
            ("ln_mlp_g", [D]), ("ln_mlp_b", [D]),
            ("Wq", [D, D]), ("bq", [D]),
            ("Wk", [D, DI]), ("bk", [D]),
            ("Wv", [D, DI]), ("bv", [D]),
            ("W_ih", [3 * D, D]), ("b_ih", [3 * D]),
            ("W_hh", [3 * D, D]), ("b_hh", [3 * D]),
            ("W1", [MLP, D]), ("b1", [MLP]),
            ("W2", [D, MLP]), ("b2", [D]),
            ("slot_init", [1, N, D]),
        ]
    }
    aug_d = nc.dram_tensor("aug", [bl, t_len, DI + D], F32, kind="ExternalOutput")
    slots_d = nc.dram_tensor("slots_out", [bl, N, D], F32, kind="ExternalOutput")

    dbg_d = {}
    if dbg:
        TCd = t_len // 128
        NBd = bl * N
        for name, shape, dt_ in [
            ("dbg_xs", [128, TCd, DI], BF16),
            ("dbg_kT", [128, 2, t_len], BF16),
            ("dbg_attn", [128, TCd, N], BF16),
            ("dbg_q0", [128, 2, 32 * bl], BF16),
            ("dbg_u00", [N, DI], F32),
            ("dbg_upd0", [32 * bl, D], F32),
            ("dbg_slots0", [32 * bl, D], F32),
            ("dbg_slotsg0", [32 * bl, D], F32),
        ]:
            dbg_d[name] = nc.dram_tensor(name, shape, dt_, kind="ExternalOutput")

    with tile.TileContext(nc) as tc:
        with ExitStack() as ctx:
            _emit(ctx, tc, nc, x_d, w, aug_d, slots_d, t_len, bl, slot_r, dbg_d)
    nc.compile()
    return nc


def _emit(ctx, tc, nc, x_d, w, aug_d, slots_d, t_len, bl, slot_r, dbg_d=None):
    P = 128
    TC = t_len // P
    TC8 = t_len // 512
    NB = bl * N          # logical stacked rows
    NP = 32 * bl         # padded: batch b at partition 32*b
    SR = F32R if slot_r else F32
    x_ap = x_d.ap()
    aug_ap = aug_d.ap()

    const = ctx.enter_context(tc.tile_pool(name="const", bufs=1))
    persist = ctx.enter_context(tc.tile_pool(name="persist", bufs=1))
    xtp = ctx.enter_context(tc.tile_pool(name="xtp", bufs=2))
    xin = ctx.enter_context(tc.tile_pool(name="xin", bufs=2))
    stats = ctx.enter_context(tc.tile_pool(name="stats", bufs=4))
    small = ctx.enter_context(tc.tile_pool(name="small", bufs=2))
    one = ctx.enter_context(tc.tile_pool(name="one", bufs=1))
    ps = ctx.enter_context(tc.tile_pool(name="ps", bufs=2, space="PSUM"))
    ps3 = ctx.enter_context(tc.tile_pool(name="ps3", bufs=3, space="PSUM"))
    ps1 = ctx.enter_context(tc.tile_pool(name="ps1", bufs=1, space="PSUM"))

    # ---------------- constants ----------------
    I128f = const.tile([P, P], F32, name="I128f")
    make_identity(nc, I128f)
    I128b = const.tile([P, P], BF16, name="I128b")
    nc.any.tensor_copy(I128b, I128f)

    eps_ln = const.tile([P, 1], F32, name="eps_ln")
    nc.vector.memset(eps_ln, EPS_LN)
    ones_col_b = const.tile([P, 1], BF16, name="ones_col_b")
    nc.vector.memset(ones_col_b, 1.0)
    ones_row_b = const.tile([1, P], BF16, name="ones_row_b")
    nc.vector.memset(ones_row_b, 1.0)

    def col2(dram1d, name):  # [256] -> [128, 2] f32, di = c*128 + p
        t_ = const.tile([P, 2], F32, name=name)
        nc.sync.dma_start(t_, dram1d.ap().rearrange("(c p) -> p c", p=P))
        return t_

    g_in = col2(w["ln_in_g"], "g_in")
    b_in = col2(w["ln_in_b"], "b_in")
    b_sl = col2(w["ln_slot_b"], "b_sl")
    g_sl = col2(w["ln_slot_g"], "g_sl")
    g_ml = col2(w["ln_mlp_g"], "g_ml")
    b_ml = col2(w["ln_mlp_b"], "b_ml")
    bk_c = col2(w["bk"], "bk_c")
    bq_c = col2(w["bq"], "bq_c")

    # bias rows kept in bf16 (only used as K=1 matmul rhs; zero in practice)
    bih_row = const.tile([1, 3 * D], BF16, name="bih_row")
    bhh_row = const.tile([1, 3 * D], BF16, name="bhh_row")
    c2_row = const.tile([1, D], BF16, name="c2_row")

    # -------- weights: load rows, PE-transpose, fold LN gammas --------
    WkgT = const.tile([P, 2, DI], BF16, name="WkgT")    # [di, d] bf16
    WqgT = const.tile([P, 2, D], BF16, name="WqgT")
    WvgT = const.tile([P, 2, D], SR, name="WvgT")      # [di, d] f32 (iter use)
    WihT = const.tile([P, 2, 3 * D], SR, name="WihT")  # [d, gate]
    WhhT = const.tile([P, 2, 3 * D], SR, name="WhhT")
    W1gT = const.tile([P, 2, MLP], SR, name="W1gT")    # [d, m]
    W2T = const.tile([P, 4, D], SR, name="W2T")        # [m, d]
    ck = const.tile([P, 2], F32, name="ck")             # b_ln@Wk.T + bk (col)
    cq16 = const.tile([P, 2], F32, name="cq16")         # (b_ln@Wq.T + bq)/16
    cv_row = const.tile([1, D], BF16, name="cv_row")    # b_ln@Wv.T + bv (row)
    c1_row = const.tile([1, MLP], BF16, name="c1_row")  # b_ln@W1.T + b1

    with tc.tile_pool(name="wsetup", bufs=1) as wsu:
        def load_rows(dram, nrow, ncol, tag="raw"):
            t_ = wsu.tile([P, nrow // P, ncol], F32, tag=tag)
            nc.sync.dma_start(t_, dram.ap().rearrange("(po pi) f -> pi po f", pi=P))
            return t_

        def row_tmp(dram1d, n, tag="brow"):
            t_ = wsu.tile([1, n], F32, tag=tag)
            nc.sync.dma_start(t_, dram1d.ap().unsqueeze(0))
            return t_

        for dst, src_d, n in [(bih_row, w["b_ih"], 3 * D),
                              (bhh_row, w["b_hh"], 3 * D),
                              (c2_row, w["b2"], D)]:
            for c0 in range(0, n, D):
                t_ = wsu.tile([1, D], F32, tag="brow")
                nc.sync.dma_start(t_, src_d.ap()[ds(c0, D)].unsqueeze(0))
                nc.vector.tensor_copy(dst[:, ds(c0, D)], t_)

        def transpose_to(dst, raw, rch, fch):
            for rc in range(rch):
                for fc in range(fch):
                    pt = ps.tile([P, P], F32, tag="ps_t")
                    nc.tensor.transpose(pt, raw[:, rc, ds(fc * P, P)], I128f)
                    nc.any.tensor_copy(dst[:, fc, ds(rc * P, P)], pt)

        def cvec_col(tmpT, bcol, badd, dst, scale=None):
            for dc in range(2):
                pc = ps1.tile([P, 1], F32, tag="ps_c")
                for dik in range(2):
                    nc.tensor.matmul(pc, lhsT=tmpT[:, dik, ds(dc * P, P)],
                                     rhs=bcol[:, dik:dik + 1],
                                     start=dik == 0, stop=dik == 1)
                if scale is None:
                    nc.vector.tensor_scalar(dst[:, dc:dc + 1], pc,
                                            badd[:, dc:dc + 1], None, op0=OP.add)
                else:
                    nc.vector.tensor_scalar(dst[:, dc:dc + 1], pc,
                                            badd[:, dc:dc + 1], scale,
                                            op0=OP.add, op1=OP.mult)

        def cvec_row(tmpT, bcol, badd_dram, dst, width):
            pc = ps1.tile([1, width], F32, tag="ps_c")
            for dik in range(2):
                nc.tensor.matmul(pc, lhsT=bcol[:, dik:dik + 1],
                                 rhs=tmpT[:, dik, :],
                                 start=dik == 0, stop=dik == 1)
            for c0 in range(0, width, D):
                t_ = wsu.tile([1, D], F32, tag="brow")
                nc.sync.dma_start(t_, badd_dram.ap()[ds(c0, D)].unsqueeze(0))
                nc.vector.tensor_tensor(dst[:, ds(c0, D)], pc[:, ds(c0, D)],
                                        t_, OP.add)

        # Wk -> WkgT bf16 + ck
        raw = load_rows(w["Wk"], D, DI)
        tmpT = wsu.tile([P, 2, D], F32, tag="tmpT")
        transpose_to(tmpT, raw, 2, 2)
        cvec_col(tmpT, b_in, bk_c, ck)
        for dik in range(2):
            nc.vector.tensor_scalar_mul(WkgT[:, dik, :], tmpT[:, dik, :],
                                        g_in[:, dik:dik + 1])
        # Wq -> WqgT bf16 + cq16
        raw = load_rows(w["Wq"], D, D)
        tmpT = wsu.tile([P, 2, D], F32, tag="tmpT")
        transpose_to(tmpT, raw, 2, 2)
        cvec_col(tmpT, b_sl, bq_c, cq16, scale=1.0 / 16.0)
        for dik in range(2):
            nc.vector.tensor_scalar_mul(WqgT[:, dik, :], tmpT[:, dik, :],
                                        g_sl[:, dik:dik + 1])
        # Wv -> WvgT f32 + cv_row
        raw = load_rows(w["Wv"], D, DI)
        tmpT = wsu.tile([P, 2, D], F32, tag="tmpT")
        transpose_to(tmpT, raw, 2, 2)
        cvec_row(tmpT, b_in, w["bv"], cv_row, D)
        for dik in range(2):
            nc.vector.tensor_scalar_mul(WvgT[:, dik, :], tmpT[:, dik, :],
                                        g_in[:, dik:dik + 1])
        # W_ih / W_hh plain transposes (halved loads to bound the raw tag)
        for name_, dstT in [("W_ih", WihT), ("W_hh", WhhT)]:
            for half in range(2):
                rawh = wsu.tile([P, 3, D], F32, tag="raw")
                nc.sync.dma_start(
                    rawh, w[name_].ap().rearrange("(po pi) f -> pi po f", pi=P)[
                        :, ds(half * 3, 3), :])
                for rc in range(3):
                    for fc in range(2):
                        pt = ps.tile([P, P], F32, tag="ps_t")
                        nc.tensor.transpose(pt, rawh[:, rc, ds(fc * P, P)], I128f)
                        nc.any.tensor_copy(
                            dstT[:, fc, ds((half * 3 + rc) * P, P)], pt)
        # W1 -> W1gT + c1
        raw = load_rows(w["W1"], MLP, D)
        tmp1 = wsu.tile([P, 2, MLP], F32, tag="tmpT")
        transpose_to(tmp1, raw, 4, 2)
        cvec_row(tmp1, b_ml, w["b1"], c1_row, MLP)
        for dik in range(2):
            nc.vector.tensor_scalar_mul(W1gT[:, dik, :], tmp1[:, dik, :],
                                        g_ml[:, dik:dik + 1])
        # W2 -> W2T
        transpose_to(W2T, load_rows(w["W2"], D, MLP), 2, 4)

    ssout = ctx.enter_context(tc.tile_pool(name="ssout", bufs=3))

    # ---------------- state ----------------
    slots = persist.tile([NP, D], F32, name="slots")
    nc.vector.memset(slots, 0.0)
    for b in range(bl):
        nc.sync.dma_start(slots[ds(32 * b, N), :], w["slot_init"].ap()[0])

    kT = [persist.tile([P, 2, t_len], BF16, name=f"kT{b}") for b in range(bl)]
    xs = [persist.tile([P, TC, DI], BF16, name=f"xs{b}") for b in range(bl)]
    attn = [persist.tile([P, TC, N], BF16, name=f"attn{b}") for b in range(bl)]

    def phase_ab(b):
        for tg in range(TC // 2):
            xg = xin.tile([P, 2, DI], F32, tag="xt")
            nc.sync.dma_start(
                xg, x_ap[b, ds(tg * 256, 256), :].rearrange(
                    "(j p) f -> p j f", p=P))
            st = stats.tile([P, 2, 6], F32, tag="st")
            for j in range(2):
                nc.vector.bn_stats(st[:, j, :], xg[:, j, :])
            mv = stats.tile([P, 2, 2], F32, tag="mv")
            for j in range(2):
                nc.vector.bn_aggr(mv[:, j, :], st[:, j, :])
            sq = stats.tile([P, 2], F32, tag="sq")
            nc.scalar.activation(sq, mv[:, :, 1], AF.Sqrt, bias=eps_ln)
            rc = stats.tile([P, 2], F32, tag="rc")
            nc.vector.reciprocal(rc, sq)
            for j in range(2):
                nc.gpsimd.tensor_scalar(xs[b][:, tg * 2 + j, :], xg[:, j, :],
                                        mv[:, j, 0:1], rc[:, j:j + 1],
                                        op0=OP.subtract, op1=OP.mult)
            nc.sync.dma_start(
                aug_ap[b, ds(tg * 256, 256), 0:DI].rearrange(
                    "(j p) f -> p j f", p=P), xg)

        for tcc in range(TC8):
            xT = xtp.tile([P, 2, 512], BF16, tag="xT")
            for c in range(2):
                pt = ps.tile([P, 512], BF16, tag="ps_t")
                for j in range(4):
                    tcx = tcc * 4 + j
                    nc.tensor.transpose(pt[:, ts(j, P)],
                                        xs[b][:, tcx, ds(c * P, P)], I128b)
                if c == 0:
                    nc.vector.tensor_copy(xT[:, c, :], pt)
                else:
                    nc.scalar.activation(xT[:, c, :], pt, AF.Copy)
            for dc in range(2):
                pk = ps.tile([P, 512], F32, tag="ps_mm")
                for dik in range(2):
                    nc.tensor.matmul(pk, lhsT=WkgT[:, dik, ds(dc * P, P)],
                                     rhs=xT[:, dik, :],
                                     start=dik == 0, stop=dik == 1)
                if dc == 0:
                    nc.vector.tensor_scalar_add(kT[b][:, dc, ts(tcc, 512)],
                                                pk, ck[:, dc:dc + 1])
                else:
                    nc.scalar.activation(kT[b][:, dc, ts(tcc, 512)], pk,
                                         AF.Identity, bias=ck[:, dc:dc + 1])


    def make_q():
        stS = stats.tile([NP, 6], F32, tag="stS")
        nc.vector.bn_stats(stS, slots)
        mvS = stats.tile([NP, 2], F32, tag="mvS")
        nc.vector.bn_aggr(mvS, stS)
        sqS = stats.tile([NP, 1], F32, tag="sqS")
        nc.scalar.activation(sqS, mvS[:, 1:2], AF.Sqrt, bias=eps_ln[:NP])
        rcS = stats.tile([NP, 1], F32, tag="rcS")
        nc.vector.reciprocal(rcS, sqS)
        ln_s = one.tile([NP, D], BF16, tag="ln_s")
        nc.vector.tensor_scalar(ln_s, slots, mvS[:, 0:1], rcS,
                                op0=OP.subtract, op1=OP.mult)
        lsT = small.tile([P, 2, NP], BF16, tag="lsT")
        for c in range(2):
            pt = ps.tile([P, NP], BF16, tag="ps_t")
            nc.tensor.transpose(pt, ln_s[:, ds(c * P, P)], I128b[:NP, :NP])
            nc.any.tensor_copy(lsT[:, c, :], pt)
        qT = small.tile([P, 2, NP], BF16, tag="qT")
        for dc in range(2):
            pq = ps.tile([P, NP], F32, tag="ps_t")
            for dik in range(2):
                nc.tensor.matmul(pq, lhsT=WqgT[:, dik, ds(dc * P, P)],
                                 rhs=lsT[:, dik, :],
                                 start=dik == 0, stop=dik == 1)
            nc.scalar.activation(qT[:, dc, :], pq, AF.Identity,
                                 bias=cq16[:, dc:dc + 1], scale=1.0 / 16.0)
        return qT

    def batch_work(b, qT, updates, dump_u0=False):
        for tg in range(TC // 4):
            pl = ps3.tile([P, 4, N], F32, tag="ps_s")
            for j in range(4):
                tcx = tg * 4 + j
                for dik in range(2):
                    nc.tensor.matmul(pl[:, j, :],
                                     lhsT=kT[b][:, dik, ts(tcx, P)],
                                     rhs=qT[:, dik, ds(32 * b, N)],
                                     start=dik == 0, stop=dik == 1)
            ae = stats.tile([P, 4, N], F32, tag="ae")
            nc.scalar.activation(ae, pl, AF.Exp)
            rs = stats.tile([P, 4], F32, tag="rs")
            nc.vector.tensor_reduce(rs, ae, axis=mybir.AxisListType.X,
                                    op=OP.add)
            rr = stats.tile([P, 4], F32, tag="rr")
            nc.vector.reciprocal(rr, rs)
            for j in range(4):
                tcx = tg * 4 + j
                nc.gpsimd.tensor_scalar_mul(attn[b][:, tcx, :],
                                            ae[:, j, :], rr[:, j:j + 1])
        pc = ps1.tile([N, 1], F32, tag="ps_c")
        for tcx in range(TC):
            nc.tensor.matmul(pc, lhsT=attn[b][:, tcx, :], rhs=ones_col_b,
                             start=tcx == 0, stop=tcx == TC - 1)
        cs = stats.tile([N, 1], F32, tag="cs")
        nc.vector.tensor_scalar(cs, pc, EPS_RENORM, None, op0=OP.add)
        wsc = stats.tile([N, 1], F32, tag="wsc")
        nc.vector.reciprocal(wsc, cs)
        pu = ps3.tile([N, DI], F32, tag="ps_s")
        for tcx in range(TC):
            nc.tensor.matmul(pu, lhsT=attn[b][:, tcx, :],
                             rhs=xs[b][:, tcx, :],
                             start=tcx == 0, stop=tcx == TC - 1)
        u0 = small.tile([N, DI], F32, tag="u0")
        nc.vector.tensor_scalar_mul(u0, pu, wsc)
        u0T = small.tile([P, 2, N], SR, tag="u0T")
        for c in range(2):
            pt = ps.tile([P, N], F32, tag="ps_t")
            nc.tensor.transpose(pt, u0[:, ds(c * P, P)], I128f[:N, :N])
            nc.any.tensor_copy(u0T[:, c, :], pt)
        pub = ps3.tile([N, D], F32, tag="ps_s")
        for dik in range(2):
            nc.tensor.matmul(pub, lhsT=u0T[:, dik, :],
                             rhs=WvgT[:, dik, :],
                             start=dik == 0, stop=False)
        nc.tensor.matmul(pub, lhsT=ones_row_b[:, :N],
                         rhs=cv_row, start=False, stop=True)
        nc.any.tensor_copy(updates[ds(32 * b, N), :], pub)
        if dump_u0 and dbg_d:
            nc.sync.dma_start(dbg_d["dbg_u00"].ap(), u0)

    def slot_update(updates, qT, it):
        # ---- GRU over stacked batches ----
        updT = small.tile([P, 2, NP], SR, tag="updT")
        slT = small.tile([P, 2, NP], SR, tag="slT")
        for c in range(2):
            pt = ps.tile([P, NP], F32, tag="ps_t")
            nc.tensor.transpose(pt, updates[:, ds(c * P, P)], I128f[:NP, :NP])
            nc.any.tensor_copy(updT[:, c, :], pt)
            pt2 = ps.tile([P, NP], F32, tag="ps_t")
            nc.tensor.transpose(pt2, slots[:, ds(c * P, P)], I128f[:NP, :NP])
            nc.any.tensor_copy(slT[:, c, :], pt2)

        prz = ps.tile([NP, 512], F32, tag="ps_mm")
        first = True
        for lhsT_t, w_t in [(updT, WihT), (slT, WhhT)]:
            for dik in range(2):
                nc.tensor.matmul(prz, lhsT=lhsT_t[:, dik, :],
                                 rhs=w_t[:, dik, 0:512],
                                 start=first, stop=False)
                first = False
        nc.tensor.matmul(prz, lhsT=ones_row_b[:, :NP], rhs=bih_row[:, 0:512],
                         start=False, stop=False)
        nc.tensor.matmul(prz, lhsT=ones_row_b[:, :NP], rhs=bhh_row[:, 0:512],
                         start=False, stop=True)

        def n_half(lhsT_t, w_t, bias_row, tag):
            pg = ps3.tile([NP, D], F32, tag=tag)
            for dik in range(2):
                nc.tensor.matmul(pg, lhsT=lhsT_t[:, dik, :],
                                 rhs=w_t[:, dik, 512:768],
                                 start=dik == 0, stop=False)
            nc.tensor.matmul(pg, lhsT=ones_row_b[:, :NP],
                             rhs=bias_row[:, 512:768], start=False, stop=True)
            return pg

        pnx = n_half(updT, WihT, bih_row, "ps_s")
        pnh = n_half(slT, WhhT, bhh_row, "ps_s")

        def tt(in0, in1, op, tag="gtmp"):
            o = small.tile([NP, D], F32, tag=tag)
            nc.vector.tensor_tensor(o, in0, in1, op)
            return o

        r_g = one.tile([NP, D], F32, tag="r_g")
        nc.scalar.activation(r_g, prz[:, 0:D], AF.Sigmoid)
        z_g = one.tile([NP, D], F32, tag="z_g")
        nc.scalar.activation(z_g, prz[:, D:2 * D], AF.Sigmoid)
        t1 = tt(r_g, pnh, OP.mult)
        t2 = tt(pnx, t1, OP.add)
        n_g = one.tile([NP, D], F32, tag="n_g")
        nc.scalar.activation(n_g, t2, AF.Tanh)
        d1 = tt(slots, n_g, OP.subtract)
        d2 = tt(z_g, d1, OP.mult)
        nc.vector.tensor_tensor(slots, n_g, d2, OP.add)
        if dbg_d and it == 0:
            nc.sync.dma_start(dbg_d["dbg_q0"].ap(), qT[:])
            nc.sync.dma_start(dbg_d["dbg_upd0"].ap(), updates)
            nc.sync.dma_start(dbg_d["dbg_slotsg0"].ap(), slots)

        # ---- MLP residual ----
        stM = stats.tile([NP, 6], F32, tag="stS")
        nc.vector.bn_stats(stM, slots)
        mvM = stats.tile([NP, 2], F32, tag="mvS")
        nc.vector.bn_aggr(mvM, stM)
        sqM = stats.tile([NP, 1], F32, tag="sqS")
        nc.scalar.activation(sqM, mvM[:, 1:2], AF.Sqrt, bias=eps_ln[:NP])
        rcM = stats.tile([NP, 1], F32, tag="rcS")
        nc.vector.reciprocal(rcM, sqM)
        ln_h = one.tile([NP, D], F32, tag="ln_h")
        nc.vector.tensor_scalar(ln_h, slots, mvM[:, 0:1], rcM,
                                op0=OP.subtract, op1=OP.mult)
        lhT = small.tile([P, 2, NP], SR, tag="lhT")
        for c in range(2):
            pt = ps.tile([P, NP], F32, tag="ps_t")
            nc.tensor.transpose(pt, ln_h[:, ds(c * P, P)], I128f[:NP, :NP])
            nc.any.tensor_copy(lhT[:, c, :], pt)
        pm1 = ps.tile([NP, MLP], F32, tag="ps_mm")
        for dik in range(2):
            nc.tensor.matmul(pm1, lhsT=lhT[:, dik, :],
                             rhs=W1gT[:, dik, :],
                             start=dik == 0, stop=False)
        nc.tensor.matmul(pm1, lhsT=ones_row_b[:, :NP],
                         rhs=c1_row, start=False, stop=True)
        h1 = one.tile([NP, MLP], F32, tag="h1")
        nc.scalar.activation(h1, pm1, AF.Relu)
        h1T = one.tile([P, 4, NP], SR, tag="h1T")
        for mc in range(4):
            pt = ps.tile([P, NP], F32, tag="ps_t")
            nc.tensor.transpose(pt, h1[:, ds(mc * P, P)], I128f[:NP, :NP])
            nc.any.tensor_copy(h1T[:, mc, :], pt)
        pm2 = ps3.tile([NP, D], F32, tag="ps_s")
        for mc in range(4):
            nc.tensor.matmul(pm2, lhsT=h1T[:, mc, :],
                             rhs=W2T[:, mc, :],
                             start=mc == 0, stop=False)
        nc.tensor.matmul(pm2, lhsT=ones_row_b[:, :NP],
                         rhs=c2_row, start=False, stop=True)
        nc.vector.tensor_tensor(slots, slots, pm2, OP.add)
        if dbg_d and it == 0:
            nc.sync.dma_start(dbg_d["dbg_slots0"].ap(), slots)

    # ---- pipelined schedule: iter-0 per-batch work rides phase A/B ----
    qT0 = make_q()
    upd0 = small.tile([NP, D], F32, tag="updates")
    nc.vector.memset(upd0, 0.0)
    for b in range(bl):
        phase_ab(b)
        if b > 0:
            batch_work(b - 1, qT0, upd0, dump_u0=(b == 1))
    batch_work(bl - 1, qT0, upd0, dump_u0=(bl == 1))
    if dbg_d:
        nc.sync.dma_start(dbg_d["dbg_xs"].ap(), xs[0][:])
        nc.sync.dma_start(dbg_d["dbg_kT"].ap(), kT[0][:])
    slot_update(upd0, qT0, 0)
    for it in range(1, NITERS):
        qTi = make_q()
        updi = small.tile([NP, D], F32, tag="updates")
        nc.vector.memset(updi, 0.0)
        for b in range(bl):
            batch_work(b, qTi, updi)
        slot_update(updi, qTi, it)

    # ---------------- tail ----------------
    for b in range(bl):
        sb16 = one.tile([N, D], BF16, tag="sb16")
        nc.any.tensor_copy(sb16, slots[ds(32 * b, N), :])
        for tg in range(TC // 2):
            pa = ps1.tile([N, 256], BF16, tag="ps_c")
            for j in range(2):
                nc.tensor.transpose(pa[:, ts(j, P)],
                                    attn[b][:, tg * 2 + j, :], I128b)
            at = small.tile([N, 256], BF16, tag="at")
            nc.vector.tensor_copy(at, pa)
            so = ssout.tile([P, 2, D], F32, tag="so")
            for j in range(2):
                pss = ps3.tile([P, D], F32, tag="ps_s")
                nc.tensor.matmul(pss, lhsT=at[:, ts(j, P)], rhs=sb16,
                                 start=True, stop=True)
                if j % 2 == 0:
                    nc.scalar.activation(so[:, j, :], pss, AF.Copy)
                else:
                    nc.vector.tensor_copy(so[:, j, :], pss)
            nc.sync.dma_start(
                aug_ap[b, ds(tg * 256, 256), DI:DI + D].rearrange(
                    "(j p) f -> p j f", p=P), so)
    for b in range(bl):
        nc.sync.dma_start(slots_d.ap()[b], slots[ds(32 * b, N), :])
    if dbg_d:
        nc.sync.dma_start(dbg_d["dbg_attn"].ap(), attn[0][:])


_NC_CACHE = {}


def get_nc(t_len=T, bl=BL, slot_r=True):
    key = (t_len, bl, slot_r)
    if key not in _NC_CACHE:
        _NC_CACHE[key] = build_kernel(t_len, bl, slot_r)
    return _NC_CACHE[key]


def kernel(**inputs):
    arrs = {k: np.ascontiguousarray(np.asarray(v, dtype=np.float32))
            for k, v in inputs.items()}
    nc = get_nc()
    x = arrs["x"]
    in_maps = []
    for c in range(NCORES):
        m = {name: arrs[name] for name in WEIGHT_NAMES}
        m["x"] = np.ascontiguousarray(x[c * BL:(c + 1) * BL])
        in_maps.append(m)
    res = run_bass_kernel_spmd(nc, in_maps, core_ids=list(range(NCORES)))
    aug = np.concatenate([r["aug"] for r in res.results], axis=0)
    slots = np.concatenate([r["slots_out"] for r in res.results], axis=0)
    return aug, slots


# revision 42
# speedup vs baseline: 28988.7395x; 1.0079x over previous
"""NamedSlotMemory Trainium2 Bass kernel.

Contract: kernel(**inputs) takes FULL (unsharded) inputs, shards batch B=32
across 8 NeuronCores (4 batches each), runs one SPMD Bass program, gathers and
returns (augmented, slots) as numpy float32.

Per-core plan (4 local batches, T=4096, DI=D=256, N=16 slots, 3 GRU iters):
  phase A: per 128-row x tile: LN stats via bn_stats; x_std=(x-m)*r -> bf16
           (kept resident, doubles as the "v source"); stream x to aug[:, :256].
  phase B: transpose x_std per 512-chunk on PE; kT[d,t] = (Wk*g_ln).T matmuls
           (bf16, LN bias folded to a per-partition ACT bias on the copy).
  iters:   logits[t,n] = kT-slices (PE weights) x qT (bf16); softmax over the
           free dim with fused Exp+row-sum; attn bf16; colsum via ones-matmul;
           updates = (attnT @ x_std) @ (Wv*g_ln).T with 1/(colsum+eps) folded
           into the PSUM->SBUF copy; GRU + LN + MLP batched over all local
           batches on [64,256] tiles in fp32/fp32r.
  tail:    soft_slot = attnT @ slots per t-tile -> aug[:, 256:]; slots out.
"""

from contextlib import ExitStack

import numpy as np

import concourse.bass as bass
import concourse.bacc as bacc
import concourse.tile as tile
from concourse import mybir
from concourse.bass import ds, ts
from concourse.bass_utils import run_bass_kernel_spmd
from concourse.masks import make_identity

F32 = mybir.dt.float32
F32R = mybir.dt.float32r
BF16 = mybir.dt.bfloat16
AF = mybir.ActivationFunctionType
OP = mybir.AluOpType

B, T, DI, D, N, NITERS, MLP = 32, 4096, 256, 256, 16, 3, 512
NCORES = 8
BL = B // NCORES
EPS_LN = 1e-5
EPS_RENORM = 1e-8

WEIGHT_NAMES = [
    "ln_in_g", "ln_in_b", "ln_slot_g", "ln_slot_b", "ln_mlp_g", "ln_mlp_b",
    "Wq", "bq", "Wk", "bk", "Wv", "bv", "W_ih", "b_ih", "W_hh", "b_hh",
    "W1", "b1", "W2", "b2", "slot_init",
]


def build_kernel(t_len=T, bl=BL, slot_r=True, dbg=False):
    nc = bacc.Bacc("TRN2", target_bir_lowering=False, debug=False,
                   num_devices=NCORES)
    x_d = nc.dram_tensor("x", [bl, t_len, DI], F32, kind="ExternalInput")
    w = {
        name: nc.dram_tensor(name, shape, F32, kind="ExternalInput")
        for name, shape in [
            ("ln_in_g", [DI]), ("ln_in_b", [DI]),
            ("ln_slot_g", [D]), ("ln_slot_b", [D]),
            ("ln_mlp_g", [D]), ("ln_mlp_b", [D]),
            ("Wq", [D, D]), ("bq", [D]),
            ("Wk", [D, DI]), ("bk", [D]),
            ("Wv", [D, DI]), ("bv", [D]),
            ("W_ih", [3 * D, D]), ("b_ih", [3 * D]),
            ("W_hh", [3 * D, D]), ("b_hh", [3 * D]),
            ("W1", [MLP, D]), ("b1", [MLP]),
            ("W2", [D, MLP]), ("b2", [D]),
            ("slot_init", [1, N, D]),
        ]
    }
    aug_d = nc.dram_tensor("aug", [bl, t_len, DI + D], F32, kind="ExternalOutput")
    slots_d = nc.dram_tensor("slots_out", [bl, N, D], F32, kind="ExternalOutput")

    dbg_d = {}
    if dbg:
        TCd = t_len // 128
        NBd = bl * N
        for name, shape, dt_ in [
            ("dbg_xs", [128, TCd, DI], BF16),
            ("dbg_kT", [128, 2, t_len], BF16),
            ("dbg_attn", [128, TCd, N], BF16),
            ("dbg_q0", [128, 2, 32 * bl], BF16),
            ("dbg_u00", [N, DI], F32),
            ("dbg_upd0", [32 * bl, D], F32),
            ("dbg_slots0", [32 * bl, D], F32),
            ("dbg_slotsg0", [32 * bl, D], F32),
        ]:
            dbg_d[name] = nc.dram_tensor(name, shape, dt_, kind="ExternalOutput")

    with tile.TileContext(nc) as tc:
        with ExitStack() as ctx:
            _emit(ctx, tc, nc, x_d, w, aug_d, slots_d, t_len, bl, slot_r, dbg_d)
    nc.compile()
    return nc


def _emit(ctx, tc, nc, x_d, w, aug_d, slots_d, t_len, bl, slot_r, dbg_d=None):
    P = 128
    TC = t_len // P
    TC8 = t_len // 512
    NB = bl * N          # logical stacked rows
    NP = 32 * bl         # padded: batch b at partition 32*b
    SR = F32R if slot_r else F32
    x_ap = x_d.ap()
    aug_ap = aug_d.ap()

    const = ctx.enter_context(tc.tile_pool(name="const", bufs=1))
    persist = ctx.enter_context(tc.tile_pool(name="persist", bufs=1))
    xtp = ctx.enter_context(tc.tile_pool(name="xtp", bufs=2))
    xin = ctx.enter_context(tc.tile_pool(name="xin", bufs=2))
    stats = ctx.enter_context(tc.tile_pool(name="stats", bufs=4))
    small = ctx.enter_context(tc.tile_pool(name="small", bufs=2))
    one = ctx.enter_context(tc.tile_pool(name="one", bufs=1))
    ps = ctx.enter_context(tc.tile_pool(name="ps", bufs=2, space="PSUM"))
    ps3 = ctx.enter_context(tc.tile_pool(name="ps3", bufs=3, space="PSUM"))
    ps1 = ctx.enter_context(tc.tile_pool(name="ps1", bufs=1, space="PSUM"))

    # ---------------- constants ----------------
    I128f = const.tile([P, P], F32, name="I128f")
    make_identity(nc, I128f)
    I128b = const.tile([P, P], BF16, name="I128b")
    nc.any.tensor_copy(I128b, I128f)

    eps_ln = const.tile([P, 1], F32, name="eps_ln")
    nc.vector.memset(eps_ln, EPS_LN)
    ones_col_b = const.tile([P, 1], BF16, name="ones_col_b")
    nc.vector.memset(ones_col_b, 1.0)
    ones_row_b = const.tile([1, P], BF16, name="ones_row_b")
    nc.vector.memset(ones_row_b, 1.0)

    def col2(dram1d, name):  # [256] -> [128, 2] f32, di = c*128 + p
        t_ = const.tile([P, 2], F32, name=name)
        nc.sync.dma_start(t_, dram1d.ap().rearrange("(c p) -> p c", p=P))
        return t_

    g_in = col2(w["ln_in_g"], "g_in")
    b_in = col2(w["ln_in_b"], "b_in")
    b_sl = col2(w["ln_slot_b"], "b_sl")
    g_sl = col2(w["ln_slot_g"], "g_sl")
    g_ml = col2(w["ln_mlp_g"], "g_ml")
    b_ml = col2(w["ln_mlp_b"], "b_ml")
    bk_c = col2(w["bk"], "bk_c")
    bq_c = col2(w["bq"], "bq_c")

    # bias rows kept in bf16 (only used as K=1 matmul rhs; zero in practice)
    bih_row = const.tile([1, 3 * D], BF16, name="bih_row")
    bhh_row = const.tile([1, 3 * D], BF16, name="bhh_row")
    c2_row = const.tile([1, D], BF16, name="c2_row")

    # -------- weights: load rows, PE-transpose, fold LN gammas --------
    WkgT = const.tile([P, 2, DI], BF16, name="WkgT")    # [di, d] bf16
    WqgT = const.tile([P, 2, D], BF16, name="WqgT")
    WvgT = const.tile([P, 2, D], SR, name="WvgT")      # [di, d] f32 (iter use)
    WihT = const.tile([P, 2, 3 * D], SR, name="WihT")  # [d, gate]
    WhhT = const.tile([P, 2, 3 * D], SR, name="WhhT")
    W1gT = const.tile([P, 2, MLP], SR, name="W1gT")    # [d, m]
    W2T = const.tile([P, 4, D], SR, name="W2T")        # [m, d]
    ck = const.tile([P, 2], F32, name="ck")             # b_ln@Wk.T + bk (col)
    cq16 = const.tile([P, 2], F32, name="cq16")         # (b_ln@Wq.T + bq)/16
    cv_row = const.tile([1, D], BF16, name="cv_row")    # b_ln@Wv.T + bv (row)
    c1_row = const.tile([1, MLP], BF16, name="c1_row")  # b_ln@W1.T + b1

    with tc.tile_pool(name="wsetup", bufs=1) as wsu:
        def load_rows(dram, nrow, ncol, tag="raw"):
            t_ = wsu.tile([P, nrow // P, ncol], F32, tag=tag)
            nc.sync.dma_start(t_, dram.ap().rearrange("(po pi) f -> pi po f", pi=P))
            return t_

        def row_tmp(dram1d, n, tag="brow"):
            t_ = wsu.tile([1, n], F32, tag=tag)
            nc.sync.dma_start(t_, dram1d.ap().unsqueeze(0))
            return t_

        for dst, src_d, n in [(bih_row, w["b_ih"], 3 * D),
                              (bhh_row, w["b_hh"], 3 * D),
                              (c2_row, w["b2"], D)]:
            for c0 in range(0, n, D):
                t_ = wsu.tile([1, D], F32, tag="brow")
                nc.sync.dma_start(t_, src_d.ap()[ds(c0, D)].unsqueeze(0))
                nc.vector.tensor_copy(dst[:, ds(c0, D)], t_)

        def transpose_to(dst, raw, rch, fch):
            for rc in range(rch):
                for fc in range(fch):
                    pt = ps.tile([P, P], F32, tag="ps_t")
                    nc.tensor.transpose(pt, raw[:, rc, ds(fc * P, P)], I128f)
                    nc.any.tensor_copy(dst[:, fc, ds(rc * P, P)], pt)

        def cvec_col(tmpT, bcol, badd, dst, scale=None):
            for dc in range(2):
                pc = ps1.tile([P, 1], F32, tag="ps_c")
                for dik in range(2):
                    nc.tensor.matmul(pc, lhsT=tmpT[:, dik, ds(dc * P, P)],
                                     rhs=bcol[:, dik:dik + 1],
                                     start=dik == 0, stop=dik == 1)
                if scale is None:
                    nc.vector.tensor_scalar(dst[:, dc:dc + 1], pc,
                                            badd[:, dc:dc + 1], None, op0=OP.add)
                else:
                    nc.vector.tensor_scalar(dst[:, dc:dc + 1], pc,
                                            badd[:, dc:dc + 1], scale,
                                            op0=OP.add, op1=OP.mult)

        def cvec_row(tmpT, bcol, badd_dram, dst, width):
            pc = ps1.tile([1, width], F32, tag="ps_c")
            for dik in range(2):
                nc.tensor.matmul(pc, lhsT=bcol[:, dik:dik + 1],
                                 rhs=tmpT[:, dik, :],
                                 start=dik == 0, stop=dik == 1)
            for c0 in range(0, width, D):
                t_ = wsu.tile([1, D], F32, tag="brow")
                nc.sync.dma_start(t_, badd_dram.ap()[ds(c0, D)].unsqueeze(0))
                nc.vector.tensor_tensor(dst[:, ds(c0, D)], pc[:, ds(c0, D)],
                                        t_, OP.add)

        # Wk -> WkgT bf16 + ck
        raw = load_rows(w["Wk"], D, DI)
        tmpT = wsu.tile([P, 2, D], F32, tag="tmpT")
        transpose_to(tmpT, raw, 2, 2)
        cvec_col(tmpT, b_in, bk_c, ck)
        for dik in range(2):
            nc.vector.tensor_scalar_mul(WkgT[:, dik, :], tmpT[:, dik, :],
                                        g_in[:, dik:dik + 1])
        # Wq -> WqgT bf16 + cq16
        raw = load_rows(w["Wq"], D, D)
        tmpT = wsu.tile([P, 2, D], F32, tag="tmpT")
        transpose_to(tmpT, raw, 2, 2)
        cvec_col(tmpT, b_sl, bq_c, cq16, scale=1.0 / 16.0)
        for dik in range(2):
            nc.vector.tensor_scalar_mul(WqgT[:, dik, :], tmpT[:, dik, :],
                                        g_sl[:, dik:dik + 1])
        # Wv -> WvgT f32 + cv_row
        raw = load_rows(w["Wv"], D, DI)
        tmpT = wsu.tile([P, 2, D], F32, tag="tmpT")
        transpose_to(tmpT, raw, 2, 2)
        cvec_row(tmpT, b_in, w["bv"], cv_row, D)
        for dik in range(2):
            nc.vector.tensor_scalar_mul(WvgT[:, dik, :], tmpT[:, dik, :],
                                        g_in[:, dik:dik + 1])
        # W_ih / W_hh plain transposes (halved loads to bound the raw tag)
        for name_, dstT in [("W_ih", WihT), ("W_hh", WhhT)]:
            for half in range(2):
                rawh = wsu.tile([P, 3, D], F32, tag="raw")
                nc.sync.dma_start(
                    rawh, w[name_].ap().rearrange("(po pi) f -> pi po f", pi=P)[
                        :, ds(half * 3, 3), :])
                for rc in range(3):
                    for fc in range(2):
                        pt = ps.tile([P, P], F32, tag="ps_t")
                        nc.tensor.transpose(pt, rawh[:, rc, ds(fc * P, P)], I128f)
                        nc.any.tensor_copy(
                            dstT[:, fc, ds((half * 3 + rc) * P, P)], pt)
        # W1 -> W1gT + c1
        raw = load_rows(w["W1"], MLP, D)
        tmp1 = wsu.tile([P, 2, MLP], F32, tag="tmpT")
        transpose_to(tmp1, raw, 4, 2)
        cvec_row(tmp1, b_ml, w["b1"], c1_row, MLP)
        for dik in range(2):
            nc.vector.tensor_scalar_mul(W1gT[:, dik, :], tmp1[:, dik, :],
                                        g_ml[:, dik:dik + 1])
        # W2 -> W2T
        transpose_to(W2T, load_rows(w["W2"], D, MLP), 2, 4)

    ssout = ctx.enter_context(tc.tile_pool(name="ssout", bufs=3))

    # ---------------- state ----------------
    slots = persist.tile([NP, D], F32, name="slots")
    nc.vector.memset(slots, 0.0)
    for b in range(bl):
        nc.sync.dma_start(slots[ds(32 * b, N), :], w["slot_init"].ap()[0])

    kT = [persist.tile([P, 2, t_len], BF16, name=f"kT{b}") for b in range(bl)]
    xs = [persist.tile([P, TC, DI], BF16, name=f"xs{b}") for b in range(bl)]
    attn = [persist.tile([P, TC, N], BF16, name=f"attn{b}") for b in range(bl)]

    def phase_ab(b):
        for tg in range(TC // 2):
            xg = xin.tile([P, 2, DI], F32, tag="xt")
            nc.gpsimd.dma_start(
                xg, x_ap[b, ds(tg * 256, 256), :].rearrange(
                    "(j p) f -> p j f", p=P))
            st = stats.tile([P, 2, 6], F32, tag="st")
            for j in range(2):
                nc.vector.bn_stats(st[:, j, :], xg[:, j, :])
            mv = stats.tile([P, 2, 2], F32, tag="mv")
            for j in range(2):
                nc.vector.bn_aggr(mv[:, j, :], st[:, j, :])
            sq = stats.tile([P, 2], F32, tag="sq")
            nc.scalar.activation(sq, mv[:, :, 1], AF.Sqrt, bias=eps_ln)
            rc = stats.tile([P, 2], F32, tag="rc")
            nc.vector.reciprocal(rc, sq)
            for j in range(2):
                nc.gpsimd.tensor_scalar(xs[b][:, tg * 2 + j, :], xg[:, j, :],
                                        mv[:, j, 0:1], rc[:, j:j + 1],
                                        op0=OP.subtract, op1=OP.mult)
            nc.sync.dma_start(
                aug_ap[b, ds(tg * 256, 256), 0:DI].rearrange(
                    "(j p) f -> p j f", p=P), xg)

        for tcc in range(TC8):
            xT = xtp.tile([P, 2, 512], BF16, tag="xT")
            for c in range(2):
                pt = ps.tile([P, 512], BF16, tag="ps_t")
                for j in range(4):
                    tcx = tcc * 4 + j
                    nc.tensor.transpose(pt[:, ts(j, P)],
                                        xs[b][:, tcx, ds(c * P, P)], I128b)
                if c == 0:
                    nc.vector.tensor_copy(xT[:, c, :], pt)
                else:
                    nc.scalar.activation(xT[:, c, :], pt, AF.Copy)
            for dc in range(2):
                pk = ps.tile([P, 512], F32, tag="ps_mm")
                for dik in range(2):
                    nc.tensor.matmul(pk, lhsT=WkgT[:, dik, ds(dc * P, P)],
                                     rhs=xT[:, dik, :],
                                     start=dik == 0, stop=dik == 1)
                if dc == 0:
                    nc.vector.tensor_scalar_add(kT[b][:, dc, ts(tcc, 512)],
                                                pk, ck[:, dc:dc + 1])
                else:
                    nc.scalar.activation(kT[b][:, dc, ts(tcc, 512)], pk,
                                         AF.Identity, bias=ck[:, dc:dc + 1])


    def make_q():
        stS = stats.tile([NP, 6], F32, tag="stS")
        nc.vector.bn_stats(stS, slots)
        mvS = stats.tile([NP, 2], F32, tag="mvS")
        nc.vector.bn_aggr(mvS, stS)
        sqS = stats.tile([NP, 1], F32, tag="sqS")
        nc.scalar.activation(sqS, mvS[:, 1:2], AF.Sqrt, bias=eps_ln[:NP])
        rcS = stats.tile([NP, 1], F32, tag="rcS")
        nc.vector.reciprocal(rcS, sqS)
        ln_s = one.tile([NP, D], BF16, tag="ln_s")
        nc.vector.tensor_scalar(ln_s, slots, mvS[:, 0:1], rcS,
                                op0=OP.subtract, op1=OP.mult)
        lsT = small.tile([P, 2, NP], BF16, tag="lsT")
        for c in range(2):
            pt = ps.tile([P, NP], BF16, tag="ps_t")
            nc.tensor.transpose(pt, ln_s[:, ds(c * P, P)], I128b[:NP, :NP])
            nc.any.tensor_copy(lsT[:, c, :], pt)
        qT = small.tile([P, 2, NP], BF16, tag="qT")
        for dc in range(2):
            pq = ps.tile([P, NP], F32, tag="ps_t")
            for dik in range(2):
                nc.tensor.matmul(pq, lhsT=WqgT[:, dik, ds(dc * P, P)],
                                 rhs=lsT[:, dik, :],
                                 start=dik == 0, stop=dik == 1)
            nc.scalar.activation(qT[:, dc, :], pq, AF.Identity,
                                 bias=cq16[:, dc:dc + 1], scale=1.0 / 16.0)
        return qT

    def batch_work(b, qT, updates, dump_u0=False):
        for tg in range(TC // 4):
            pl = ps3.tile([P, 4, N], F32, tag="ps_s")
            for j in range(4):
                tcx = tg * 4 + j
                for dik in range(2):
                    nc.tensor.matmul(pl[:, j, :],
                                     lhsT=kT[b][:, dik, ts(tcx, P)],
                                     rhs=qT[:, dik, ds(32 * b, N)],
                                     start=dik == 0, stop=dik == 1)
            ae = stats.tile([P, 4, N], F32, tag="ae")
            nc.scalar.activation(ae, pl, AF.Exp)
            rs = stats.tile([P, 4], F32, tag="rs")
            nc.vector.tensor_reduce(rs, ae, axis=mybir.AxisListType.X,
                                    op=OP.add)
            rr = stats.tile([P, 4], F32, tag="rr")
            nc.vector.reciprocal(rr, rs)
            for j in range(4):
                tcx = tg * 4 + j
                nc.gpsimd.tensor_scalar_mul(attn[b][:, tcx, :],
                                            ae[:, j, :], rr[:, j:j + 1])
        pc = ps1.tile([N, 1], F32, tag="ps_c")
        for tcx in range(TC):
            nc.tensor.matmul(pc, lhsT=attn[b][:, tcx, :], rhs=ones_col_b,
                             start=tcx == 0, stop=tcx == TC - 1)
        cs = stats.tile([N, 1], F32, tag="cs")
        nc.vector.tensor_scalar(cs, pc, EPS_RENORM, None, op0=OP.add)
        wsc = stats.tile([N, 1], F32, tag="wsc")
        nc.vector.reciprocal(wsc, cs)
        pu = ps3.tile([N, DI], F32, tag="ps_s")
        for tcx in range(TC):
            nc.tensor.matmul(pu, lhsT=attn[b][:, tcx, :],
                             rhs=xs[b][:, tcx, :],
                             start=tcx == 0, stop=tcx == TC - 1)
        u0 = small.tile([N, DI], F32, tag="u0")
        nc.vector.tensor_scalar_mul(u0, pu, wsc)
        u0T = small.tile([P, 2, N], SR, tag="u0T")
        for c in range(2):
            pt = ps.tile([P, N], F32, tag="ps_t")
            nc.tensor.transpose(pt, u0[:, ds(c * P, P)], I128f[:N, :N])
            nc.any.tensor_copy(u0T[:, c, :], pt)
        pub = ps3.tile([N, D], F32, tag="ps_s")
        for dik in range(2):
            nc.tensor.matmul(pub, lhsT=u0T[:, dik, :],
                             rhs=WvgT[:, dik, :],
                             start=dik == 0, stop=False)
        nc.tensor.matmul(pub, lhsT=ones_row_b[:, :N],
                         rhs=cv_row, start=False, stop=True)
        nc.any.tensor_copy(updates[ds(32 * b, N), :], pub)
        if dump_u0 and dbg_d:
            nc.sync.dma_start(dbg_d["dbg_u00"].ap(), u0)

    def slot_update(updates, qT, it):
        # ---- GRU over stacked batches ----
        updT = small.tile([P, 2, NP], SR, tag="updT")
        slT = small.tile([P, 2, NP], SR, tag="slT")
        for c in range(2):
            pt = ps.tile([P, NP], F32, tag="ps_t")
            nc.tensor.transpose(pt, updates[:, ds(c * P, P)], I128f[:NP, :NP])
            nc.any.tensor_copy(updT[:, c, :], pt)
            pt2 = ps.tile([P, NP], F32, tag="ps_t")
            nc.tensor.transpose(pt2, slots[:, ds(c * P, P)], I128f[:NP, :NP])
            nc.any.tensor_copy(slT[:, c, :], pt2)

        prz = ps.tile([NP, 512], F32, tag="ps_mm")
        first = True
        for lhsT_t, w_t in [(updT, WihT), (slT, WhhT)]:
            for dik in range(2):
                nc.tensor.matmul(prz, lhsT=lhsT_t[:, dik, :],
                                 rhs=w_t[:, dik, 0:512],
                                 start=first, stop=False)
                first = False
        nc.tensor.matmul(prz, lhsT=ones_row_b[:, :NP], rhs=bih_row[:, 0:512],
                         start=False, stop=False)
        nc.tensor.matmul(prz, lhsT=ones_row_b[:, :NP], rhs=bhh_row[:, 0:512],
                         start=False, stop=True)

        def n_half(lhsT_t, w_t, bias_row, tag):
            pg = ps3.tile([NP, D], F32, tag=tag)
            for dik in range(2):
                nc.tensor.matmul(pg, lhsT=lhsT_t[:, dik, :],
                                 rhs=w_t[:, dik, 512:768],
                                 start=dik == 0, stop=False)
            nc.tensor.matmul(pg, lhsT=ones_row_b[:, :NP],
                             rhs=bias_row[:, 512:768], start=False, stop=True)
            return pg

        pnx = n_half(updT, WihT, bih_row, "ps_s")
        pnh = n_half(slT, WhhT, bhh_row, "ps_s")

        def tt(in0, in1, op, tag="gtmp"):
            o = small.tile([NP, D], F32, tag=tag)
            nc.vector.tensor_tensor(o, in0, in1, op)
            return o

        r_g = one.tile([NP, D], F32, tag="r_g")
        nc.scalar.activation(r_g, prz[:, 0:D], AF.Sigmoid)
        z_g = one.tile([NP, D], F32, tag="z_g")
        nc.scalar.activation(z_g, prz[:, D:2 * D], AF.Sigmoid)
        t1 = tt(r_g, pnh, OP.mult)
        t2 = tt(pnx, t1, OP.add)
        n_g = one.tile([NP, D], F32, tag="n_g")
        nc.scalar.activation(n_g, t2, AF.Tanh)
        d1 = tt(slots, n_g, OP.subtract)
        d2 = tt(z_g, d1, OP.mult)
        nc.vector.tensor_tensor(slots, n_g, d2, OP.add)
        if dbg_d and it == 0:
            nc.sync.dma_start(dbg_d["dbg_q0"].ap(), qT[:])
            nc.sync.dma_start(dbg_d["dbg_upd0"].ap(), updates)
            nc.sync.dma_start(dbg_d["dbg_slotsg0"].ap(), slots)

        # ---- MLP residual ----
        stM = stats.tile([NP, 6], F32, tag="stS")
        nc.vector.bn_stats(stM, slots)
        mvM = stats.tile([NP, 2], F32, tag="mvS")
        nc.vector.bn_aggr(mvM, stM)
        sqM = stats.tile([NP, 1], F32, tag="sqS")
        nc.scalar.activation(sqM, mvM[:, 1:2], AF.Sqrt, bias=eps_ln[:NP])
        rcM = stats.tile([NP, 1], F32, tag="rcS")
        nc.vector.reciprocal(rcM, sqM)
        ln_h = one.tile([NP, D], F32, tag="ln_h")
        nc.vector.tensor_scalar(ln_h, slots, mvM[:, 0:1], rcM,
                                op0=OP.subtract, op1=OP.mult)
        lhT = small.tile([P, 2, NP], SR, tag="lhT")
        for c in range(2):
            pt = ps.tile([P, NP], F32, tag="ps_t")
            nc.tensor.transpose(pt, ln_h[:, ds(c * P, P)], I128f[:NP, :NP])
            nc.any.tensor_copy(lhT[:, c, :], pt)
        pm1 = ps.tile([NP, MLP], F32, tag="ps_mm")
        for dik in range(2):
            nc.tensor.matmul(pm1, lhsT=lhT[:, dik, :],
                             rhs=W1gT[:, dik, :],
                             start=dik == 0, stop=False)
        nc.tensor.matmul(pm1, lhsT=ones_row_b[:, :NP],
                         rhs=c1_row, start=False, stop=True)
        h1 = one.tile([NP, MLP], F32, tag="h1")
        nc.scalar.activation(h1, pm1, AF.Relu)
        h1T = one.tile([P, 4, NP], SR, tag="h1T")
        for mc in range(4):
            pt = ps.tile([P, NP], F32, tag="ps_t")
            nc.tensor.transpose(pt, h1[:, ds(mc * P, P)], I128f[:NP, :NP])
            nc.any.tensor_copy(h1T[:, mc, :], pt)
        pm2 = ps3.tile([NP, D], F32, tag="ps_s")
        for mc in range(4):
            nc.tensor.matmul(pm2, lhsT=h1T[:, mc, :],
                             rhs=W2T[:, mc, :],
                             start=mc == 0, stop=False)
        nc.tensor.matmul(pm2, lhsT=ones_row_b[:, :NP],
                         rhs=c2_row, start=False, stop=True)
        nc.vector.tensor_tensor(slots, slots, pm2, OP.add)
        if dbg_d and it == 0:
            nc.sync.dma_start(dbg_d["dbg_slots0"].ap(), slots)

    # ---- pipelined schedule: iter-0 per-batch work rides phase A/B ----
    qT0 = make_q()
    upd0 = small.tile([NP, D], F32, tag="updates")
    nc.vector.memset(upd0, 0.0)
    for b in range(bl):
        phase_ab(b)
        if b > 0:
            batch_work(b - 1, qT0, upd0, dump_u0=(b == 1))
    batch_work(bl - 1, qT0, upd0, dump_u0=(bl == 1))
    if dbg_d:
        nc.sync.dma_start(dbg_d["dbg_xs"].ap(), xs[0][:])
        nc.sync.dma_start(dbg_d["dbg_kT"].ap(), kT[0][:])
    slot_update(upd0, qT0, 0)
    for it in range(1, NITERS):
        qTi = make_q()
        updi = small.tile([NP, D], F32, tag="updates")
        nc.vector.memset(updi, 0.0)
        for b in range(bl):
            batch_work(b, qTi, updi)
        slot_update(updi, qTi, it)

    # ---------------- tail ----------------
    for b in range(bl):
        sb16 = one.tile([N, D], BF16, tag="sb16")
        nc.any.tensor_copy(sb16, slots[ds(32 * b, N), :])
        for tg in range(TC // 2):
            pa = ps1.tile([N, 256], BF16, tag="ps_c")
            for j in range(2):
                nc.tensor.transpose(pa[:, ts(j, P)],
                                    attn[b][:, tg * 2 + j, :], I128b)
            at = small.tile([N, 256], BF16, tag="at")
            nc.vector.tensor_copy(at, pa)
            so = ssout.tile([P, 2, D], F32, tag="so")
            for j in range(2):
                pss = ps3.tile([P, D], F32, tag="ps_s")
                nc.tensor.matmul(pss, lhsT=at[:, ts(j, P)], rhs=sb16,
                                 start=True, stop=True)
                if j % 2 == 0:
                    nc.scalar.activation(so[:, j, :], pss, AF.Copy)
                else:
                    nc.vector.tensor_copy(so[:, j, :], pss)
            nc.gpsimd.dma_start(
                aug_ap[b, ds(tg * 256, 256), DI:DI + D].rearrange(
                    "(j p) f -> p j f", p=P), so)
    for b in range(bl):
        nc.sync.dma_start(slots_d.ap()[b], slots[ds(32 * b, N), :])
    if dbg_d:
        nc.sync.dma_start(dbg_d["dbg_attn"].ap(), attn[0][:])


_NC_CACHE = {}


def get_nc(t_len=T, bl=BL, slot_r=True):
    key = (t_len, bl, slot_r)
    if key not in _NC_CACHE:
        _NC_CACHE[key] = build_kernel(t_len, bl, slot_r)
    return _NC_CACHE[key]


def kernel(**inputs):
    arrs = {k: np.ascontiguousarray(np.asarray(v, dtype=np.float32))
            for k, v in inputs.items()}
    nc = get_nc()
    x = arrs["x"]
    in_maps = []
    for c in range(NCORES):
        m = {name: arrs[name] for name in WEIGHT_NAMES}
        m["x"] = np.ascontiguousarray(x[c * BL:(c + 1) * BL])
        in_maps.append(m)
    res = run_bass_kernel_spmd(nc, in_maps, core_ids=list(range(NCORES)))
    aug = np.concatenate([r["aug"] for r in res.results], axis=0)
    slots = np.concatenate([r["slots_out"] for r in res.results], axis=0)
    return aug, slots


# revision 53
# speedup vs baseline: 29986.0455x; 1.0344x over previous
"""NamedSlotMemory Trainium2 Bass kernel.

Contract: kernel(**inputs) takes FULL (unsharded) inputs, shards batch B=32
across 8 NeuronCores (4 batches each), runs one SPMD Bass program, gathers and
returns (augmented, slots) as numpy float32.

Per-core plan (4 local batches, T=4096, DI=D=256, N=16 slots, 3 GRU iters):
  phase A: per 128-row x tile: LN stats via bn_stats; x_std=(x-m)*r -> bf16
           (kept resident, doubles as the "v source"); stream x to aug[:, :256].
  phase B: transpose x_std per 512-chunk on PE; kT[d,t] = (Wk*g_ln).T matmuls
           (bf16, LN bias folded to a per-partition ACT bias on the copy).
  iters:   logits[t,n] = kT-slices (PE weights) x qT (bf16); softmax over the
           free dim with fused Exp+row-sum; attn bf16; colsum via ones-matmul;
           updates = (attnT @ x_std) @ (Wv*g_ln).T with 1/(colsum+eps) folded
           into the PSUM->SBUF copy; GRU + LN + MLP batched over all local
           batches on [64,256] tiles in fp32/fp32r.
  tail:    soft_slot = attnT @ slots per t-tile -> aug[:, 256:]; slots out.
"""

from contextlib import ExitStack

import numpy as np

import concourse.bass as bass
import concourse.bacc as bacc
import concourse.tile as tile
from concourse import mybir
from concourse.bass import ds, ts
from concourse.bass_utils import run_bass_kernel_spmd
from concourse.masks import make_identity

F32 = mybir.dt.float32
F32R = mybir.dt.float32r
BF16 = mybir.dt.bfloat16
AF = mybir.ActivationFunctionType
OP = mybir.AluOpType

B, T, DI, D, N, NITERS, MLP = 32, 4096, 256, 256, 16, 3, 512
NCORES = 8
BL = B // NCORES
EPS_LN = 1e-5
EPS_RENORM = 1e-8

WEIGHT_NAMES = [
    "ln_in_g", "ln_in_b", "ln_slot_g", "ln_slot_b", "ln_mlp_g", "ln_mlp_b",
    "Wq", "bq", "Wk", "bk", "Wv", "bv", "W_ih", "b_ih", "W_hh", "b_hh",
    "W1", "b1", "W2", "b2", "slot_init",
]


def build_kernel(t_len=T, bl=BL, slot_r=True, dbg=False):
    nc = bacc.Bacc("TRN2", target_bir_lowering=False, debug=False,
                   num_devices=NCORES)
    x_d = nc.dram_tensor("x", [bl, t_len, DI], F32, kind="ExternalInput")
    w = {
        name: nc.dram_tensor(name, shape, F32, kind="ExternalInput")
        for name, shape in [
            ("ln_in_g", [DI]), ("ln_in_b", [DI]),
            ("ln_slot_g", [D]), ("ln_slot_b", [D]),
            ("ln_mlp_g", [D]), ("ln_mlp_b", [D]),
            ("Wq", [D, D]), ("bq", [D]),
            ("Wk", [D, DI]), ("bk", [D]),
            ("Wv", [D, DI]), ("bv", [D]),
            ("W_ih", [3 * D, D]), ("b_ih", [3 * D]),
            ("W_hh", [3 * D, D]), ("b_hh", [3 * D]),
            ("W1", [MLP, D]), ("b1", [MLP]),
            ("W2", [D, MLP]), ("b2", [D]),
            ("slot_init", [1, N, D]),
        ]
    }
    aug_d = nc.dram_tensor("aug", [bl, t_len, DI + D], F32, kind="ExternalOutput")
    slots_d = nc.dram_tensor("slots_out", [bl, N, D], F32, kind="ExternalOutput")

    dbg_d = {}
    if dbg:
        TCd = t_len // 128
        NBd = bl * N
        for name, shape, dt_ in [
            ("dbg_xs", [128, TCd, DI], BF16),
            ("dbg_kT", [128, 2, t_len], BF16),
            ("dbg_attn", [128, TCd, N], BF16),
            ("dbg_q0", [128, 2, 32 * bl], BF16),
            ("dbg_u00", [N, DI], F32),
            ("dbg_upd0", [32 * bl, D], F32),
            ("dbg_slots0", [32 * bl, D], F32),
            ("dbg_slotsg0", [32 * bl, D], F32),
        ]:
            dbg_d[name] = nc.dram_tensor(name, shape, dt_, kind="ExternalOutput")

    with tile.TileContext(nc) as tc:
        with ExitStack() as ctx:
            _emit(ctx, tc, nc, x_d, w, aug_d, slots_d, t_len, bl, slot_r, dbg_d)
    nc.compile()
    return nc


def _emit(ctx, tc, nc, x_d, w, aug_d, slots_d, t_len, bl, slot_r, dbg_d=None):
    P = 128
    TC = t_len // P
    TC8 = t_len // 512
    NB = bl * N          # logical stacked rows
    NP = 32 * bl         # padded: batch b at partition 32*b
    SR = F32R if slot_r else F32
    x_ap = x_d.ap()
    aug_ap = aug_d.ap()

    const = ctx.enter_context(tc.tile_pool(name="const", bufs=1))
    persist = ctx.enter_context(tc.tile_pool(name="persist", bufs=1))
    xtp = ctx.enter_context(tc.tile_pool(name="xtp", bufs=2))
    xin = ctx.enter_context(tc.tile_pool(name="xin", bufs=2))
    stats = ctx.enter_context(tc.tile_pool(name="stats", bufs=4))
    small = ctx.enter_context(tc.tile_pool(name="small", bufs=2))
    one = ctx.enter_context(tc.tile_pool(name="one", bufs=1))
    ps = ctx.enter_context(tc.tile_pool(name="ps", bufs=2, space="PSUM"))
    ps3 = ctx.enter_context(tc.tile_pool(name="ps3", bufs=3, space="PSUM"))
    ps1 = ctx.enter_context(tc.tile_pool(name="ps1", bufs=1, space="PSUM"))

    # ---------------- constants ----------------
    I128f = const.tile([P, P], F32, name="I128f")
    make_identity(nc, I128f)
    I128b = const.tile([P, P], BF16, name="I128b")
    nc.any.tensor_copy(I128b, I128f)

    eps_ln = const.tile([P, 1], F32, name="eps_ln")
    nc.vector.memset(eps_ln, EPS_LN)
    ones_col_b = const.tile([P, 1], BF16, name="ones_col_b")
    nc.vector.memset(ones_col_b, 1.0)
    ones_row_b = const.tile([1, P], BF16, name="ones_row_b")
    nc.vector.memset(ones_row_b, 1.0)

    def col2(dram1d, name):  # [256] -> [128, 2] f32, di = c*128 + p
        t_ = const.tile([P, 2], F32, name=name)
        nc.sync.dma_start(t_, dram1d.ap().rearrange("(c p) -> p c", p=P))
        return t_

    g_in = col2(w["ln_in_g"], "g_in")
    b_in = col2(w["ln_in_b"], "b_in")
    b_sl = col2(w["ln_slot_b"], "b_sl")
    g_sl = col2(w["ln_slot_g"], "g_sl")
    g_ml = col2(w["ln_mlp_g"], "g_ml")
    b_ml = col2(w["ln_mlp_b"], "b_ml")
    bk_c = col2(w["bk"], "bk_c")
    bq_c = col2(w["bq"], "bq_c")

    # bias rows kept in bf16 (only used as K=1 matmul rhs; zero in practice)
    bih_row = const.tile([1, 3 * D], BF16, name="bih_row")
    bhh_row = const.tile([1, 3 * D], BF16, name="bhh_row")
    c2_row = const.tile([1, D], BF16, name="c2_row")

    # -------- weights: load rows, PE-transpose, fold LN gammas --------
    WkgT = const.tile([P, 2, DI], BF16, name="WkgT")    # [di, d] bf16
    WqgT = const.tile([P, 2, D], BF16, name="WqgT")
    WvgT = const.tile([P, 2, D], SR, name="WvgT")      # [di, d] f32 (iter use)
    WihT = const.tile([P, 2, 3 * D], SR, name="WihT")  # [d, gate]
    WhhT = const.tile([P, 2, 3 * D], SR, name="WhhT")
    W1gT = const.tile([P, 2, MLP], SR, name="W1gT")    # [d, m]
    W2T = const.tile([P, 4, D], SR, name="W2T")        # [m, d]
    ck = const.tile([P, 2], F32, name="ck")             # b_ln@Wk.T + bk (col)
    cq16 = const.tile([P, 2], F32, name="cq16")         # (b_ln@Wq.T + bq)/16
    cv_row = const.tile([1, D], BF16, name="cv_row")    # b_ln@Wv.T + bv (row)
    c1_row = const.tile([1, MLP], BF16, name="c1_row")  # b_ln@W1.T + b1

    with tc.tile_pool(name="wsetup", bufs=1) as wsu:
        def load_rows(dram, nrow, ncol, tag="raw"):
            t_ = wsu.tile([P, nrow // P, ncol], F32, tag=tag)
            nc.sync.dma_start(t_, dram.ap().rearrange("(po pi) f -> pi po f", pi=P))
            return t_

        def row_tmp(dram1d, n, tag="brow"):
            t_ = wsu.tile([1, n], F32, tag=tag)
            nc.sync.dma_start(t_, dram1d.ap().unsqueeze(0))
            return t_

        for dst, src_d, n in [(bih_row, w["b_ih"], 3 * D),
                              (bhh_row, w["b_hh"], 3 * D),
                              (c2_row, w["b2"], D)]:
            for c0 in range(0, n, D):
                t_ = wsu.tile([1, D], F32, tag="brow")
                nc.sync.dma_start(t_, src_d.ap()[ds(c0, D)].unsqueeze(0))
                nc.vector.tensor_copy(dst[:, ds(c0, D)], t_)

        def transpose_to(dst, raw, rch, fch):
            for rc in range(rch):
                for fc in range(fch):
                    pt = ps.tile([P, P], F32, tag="ps_t")
                    nc.tensor.transpose(pt, raw[:, rc, ds(fc * P, P)], I128f)
                    nc.any.tensor_copy(dst[:, fc, ds(rc * P, P)], pt)

        def cvec_col(tmpT, bcol, badd, dst, scale=None):
            for dc in range(2):
                pc = ps1.tile([P, 1], F32, tag="ps_c")
                for dik in range(2):
                    nc.tensor.matmul(pc, lhsT=tmpT[:, dik, ds(dc * P, P)],
                                     rhs=bcol[:, dik:dik + 1],
                                     start=dik == 0, stop=dik == 1)
                if scale is None:
                    nc.vector.tensor_scalar(dst[:, dc:dc + 1], pc,
                                            badd[:, dc:dc + 1], None, op0=OP.add)
                else:
                    nc.vector.tensor_scalar(dst[:, dc:dc + 1], pc,
                                            badd[:, dc:dc + 1], scale,
                                            op0=OP.add, op1=OP.mult)

        def cvec_row(tmpT, bcol, badd_dram, dst, width):
            pc = ps1.tile([1, width], F32, tag="ps_c")
            for dik in range(2):
                nc.tensor.matmul(pc, lhsT=bcol[:, dik:dik + 1],
                                 rhs=tmpT[:, dik, :],
                                 start=dik == 0, stop=dik == 1)
            for c0 in range(0, width, D):
                t_ = wsu.tile([1, D], F32, tag="brow")
                nc.sync.dma_start(t_, badd_dram.ap()[ds(c0, D)].unsqueeze(0))
                nc.vector.tensor_tensor(dst[:, ds(c0, D)], pc[:, ds(c0, D)],
                                        t_, OP.add)

        # Wk -> WkgT bf16 + ck
        raw = load_rows(w["Wk"], D, DI)
        tmpT = wsu.tile([P, 2, D], F32, tag="tmpT")
        transpose_to(tmpT, raw, 2, 2)
        cvec_col(tmpT, b_in, bk_c, ck)
        for dik in range(2):
            nc.vector.tensor_scalar_mul(WkgT[:, dik, :], tmpT[:, dik, :],
                                        g_in[:, dik:dik + 1])
        # Wq -> WqgT bf16 + cq16
        raw = load_rows(w["Wq"], D, D)
        tmpT = wsu.tile([P, 2, D], F32, tag="tmpT")
        transpose_to(tmpT, raw, 2, 2)
        cvec_col(tmpT, b_sl, bq_c, cq16, scale=1.0 / 16.0)
        for dik in range(2):
            nc.vector.tensor_scalar_mul(WqgT[:, dik, :], tmpT[:, dik, :],
                                        g_sl[:, dik:dik + 1])
        # Wv -> WvgT f32 + cv_row
        raw = load_rows(w["Wv"], D, DI)
        tmpT = wsu.tile([P, 2, D], F32, tag="tmpT")
        transpose_to(tmpT, raw, 2, 2)
        cvec_row(tmpT, b_in, w["bv"], cv_row, D)
        for dik in range(2):
            nc.vector.tensor_scalar_mul(WvgT[:, dik, :], tmpT[:, dik, :],
                                        g_in[:, dik:dik + 1])
        # W_ih / W_hh plain transposes (halved loads to bound the raw tag)
        for name_, dstT in [("W_ih", WihT), ("W_hh", WhhT)]:
            for half in range(2):
                rawh = wsu.tile([P, 3, D], F32, tag="raw")
                nc.sync.dma_start(
                    rawh, w[name_].ap().rearrange("(po pi) f -> pi po f", pi=P)[
                        :, ds(half * 3, 3), :])
                for rc in range(3):
                    for fc in range(2):
                        pt = ps.tile([P, P], F32, tag="ps_t")
                        nc.tensor.transpose(pt, rawh[:, rc, ds(fc * P, P)], I128f)
                        nc.any.tensor_copy(
                            dstT[:, fc, ds((half * 3 + rc) * P, P)], pt)
        # W1 -> W1gT + c1
        raw = load_rows(w["W1"], MLP, D)
        tmp1 = wsu.tile([P, 2, MLP], F32, tag="tmpT")
        transpose_to(tmp1, raw, 4, 2)
        cvec_row(tmp1, b_ml, w["b1"], c1_row, MLP)
        for dik in range(2):
            nc.vector.tensor_scalar_mul(W1gT[:, dik, :], tmp1[:, dik, :],
                                        g_ml[:, dik:dik + 1])
        # W2 -> W2T
        transpose_to(W2T, load_rows(w["W2"], D, MLP), 2, 4)

    ssout = ctx.enter_context(tc.tile_pool(name="ssout", bufs=3))

    # ---------------- state ----------------
    slots = persist.tile([NP, D], F32, name="slots")
    nc.vector.memset(slots, 0.0)
    for b in range(bl):
        nc.sync.dma_start(slots[ds(32 * b, N), :], w["slot_init"].ap()[0])

    kT = [persist.tile([P, 2, t_len], BF16, name=f"kT{b}") for b in range(bl)]
    xs = [persist.tile([P, TC, DI + 1], BF16, name=f"xs{b}") for b in range(bl)]
    for b in range(bl):
        nc.gpsimd.memset(xs[b][:, :, DI:DI + 1], 1.0)
    attn = [persist.tile([P, TC, N], BF16, name=f"attn{b}") for b in range(bl)]

    def phase_ab(b):
        for tg in range(TC // 2):
            xg = xin.tile([P, 2, DI], F32, tag="xt")
            (nc.gpsimd if tg % 2 == 0 else nc.sync).dma_start(
                xg, x_ap[b, ds(tg * 256, 256), :].rearrange(
                    "(j p) f -> p j f", p=P))
            st = stats.tile([P, 2, 6], F32, tag="st")
            for j in range(2):
                nc.vector.bn_stats(st[:, j, :], xg[:, j, :])
            mv = stats.tile([P, 2, 2], F32, tag="mv")
            for j in range(2):
                nc.vector.bn_aggr(mv[:, j, :], st[:, j, :])
            sq = stats.tile([P, 2], F32, tag="sq")
            nc.scalar.activation(sq, mv[:, :, 1], AF.Sqrt, bias=eps_ln)
            rc = stats.tile([P, 2], F32, tag="rc")
            nc.vector.reciprocal(rc, sq)
            nc.sync.dma_start(
                aug_ap[b, ds(tg * 256, 256), 0:DI].rearrange(
                    "(j p) f -> p j f", p=P), xg)
            for j in range(2):
                nc.gpsimd.tensor_scalar(xs[b][:, tg * 2 + j, 0:DI],
                                        xg[:, j, :],
                                        mv[:, j, 0:1], rc[:, j:j + 1],
                                        op0=OP.subtract, op1=OP.mult)

        for tcc in range(TC8):
            xT = xtp.tile([P, 2, 512], BF16, tag="xT")
            for c in range(2):
                pt = ps.tile([P, 512], BF16, tag="ps_t")
                for j in range(4):
                    tcx = tcc * 4 + j
                    nc.tensor.transpose(pt[:, ts(j, P)],
                                        xs[b][:, tcx, ds(c * P, P)], I128b)
                nc.vector.tensor_copy(xT[:, c, :], pt)
            for dc in range(2):
                pk = ps.tile([P, 512], F32, tag="ps_mm")
                for dik in range(2):
                    nc.tensor.matmul(pk, lhsT=WkgT[:, dik, ds(dc * P, P)],
                                     rhs=xT[:, dik, :],
                                     start=dik == 0, stop=dik == 1)
                nc.scalar.activation(kT[b][:, dc, ts(tcc, 512)], pk,
                                     AF.Identity, bias=ck[:, dc:dc + 1])


    def make_q():
        stS = stats.tile([NP, 6], F32, tag="stS")
        nc.vector.bn_stats(stS, slots)
        mvS = stats.tile([NP, 2], F32, tag="mvS")
        nc.vector.bn_aggr(mvS, stS)
        sqS = stats.tile([NP, 1], F32, tag="sqS")
        nc.scalar.activation(sqS, mvS[:, 1:2], AF.Sqrt, bias=eps_ln[:NP])
        rcS = stats.tile([NP, 1], F32, tag="rcS")
        nc.vector.reciprocal(rcS, sqS)
        ln_s = one.tile([NP, D], BF16, tag="ln_s")
        nc.vector.tensor_scalar(ln_s, slots, mvS[:, 0:1], rcS,
                                op0=OP.subtract, op1=OP.mult)
        lsT = small.tile([P, 2, NP], BF16, tag="lsT")
        for c in range(2):
            pt = ps.tile([P, NP], BF16, tag="ps_t")
            nc.tensor.transpose(pt, ln_s[:, ds(c * P, P)], I128b[:NP, :NP])
            nc.any.tensor_copy(lsT[:, c, :], pt)
        qT = small.tile([P, 2, NP], BF16, tag="qT")
        for dc in range(2):
            pq = ps.tile([P, NP], F32, tag="ps_t")
            for dik in range(2):
                nc.tensor.matmul(pq, lhsT=WqgT[:, dik, ds(dc * P, P)],
                                 rhs=lsT[:, dik, :],
                                 start=dik == 0, stop=dik == 1)
            nc.scalar.activation(qT[:, dc, :], pq, AF.Identity,
                                 bias=cq16[:, dc:dc + 1], scale=1.0 / 16.0)
        return qT

    def batch_work(b, qT, updates, dump_u0=False):
        for tg in range(TC // 4):
            pl = ps3.tile([P, 4, N], F32, tag="ps_s")
            for j in range(4):
                tcx = tg * 4 + j
                for dik in range(2):
                    nc.tensor.matmul(pl[:, j, :],
                                     lhsT=kT[b][:, dik, ts(tcx, P)],
                                     rhs=qT[:, dik, ds(32 * b, N)],
                                     start=dik == 0, stop=dik == 1)
            ae = stats.tile([P, 4, N], F32, tag="ae")
            nc.scalar.activation(ae, pl, AF.Exp)
            rs = stats.tile([P, 4], F32, tag="rs")
            nc.vector.tensor_reduce(rs, ae, axis=mybir.AxisListType.X,
                                    op=OP.add)
            rr = stats.tile([P, 4], F32, tag="rr")
            nc.vector.reciprocal(rr, rs)
            for j in range(4):
                tcx = tg * 4 + j
                nc.gpsimd.tensor_scalar_mul(attn[b][:, tcx, :],
                                            ae[:, j, :], rr[:, j:j + 1])
        pu = ps3.tile([N, DI + 1], F32, tag="ps_s")
        for tcx in range(TC):
            nc.tensor.matmul(pu, lhsT=attn[b][:, tcx, :],
                             rhs=xs[b][:, tcx, :],
                             start=tcx == 0, stop=tcx == TC - 1)
        cs = stats.tile([N, 1], F32, tag="cs")
        nc.vector.tensor_scalar(cs, pu[:, DI:DI + 1], EPS_RENORM, None,
                                op0=OP.add)
        wsc = stats.tile([N, 1], F32, tag="wsc")
        nc.vector.reciprocal(wsc, cs)
        u0 = small.tile([N, DI], F32, tag="u0")
        nc.vector.tensor_scalar_mul(u0, pu[:, 0:DI], wsc)
        u0T = small.tile([P, 2, N], SR, tag="u0T")
        for c in range(2):
            pt = ps.tile([P, N], F32, tag="ps_t")
            nc.tensor.transpose(pt, u0[:, ds(c * P, P)], I128f[:N, :N])
            nc.any.tensor_copy(u0T[:, c, :], pt)
        pub = ps3.tile([N, D], F32, tag="ps_s")
        for dik in range(2):
            nc.tensor.matmul(pub, lhsT=u0T[:, dik, :],
                             rhs=WvgT[:, dik, :],
                             start=dik == 0, stop=False)
        nc.tensor.matmul(pub, lhsT=ones_row_b[:, :N],
                         rhs=cv_row, start=False, stop=True)
        nc.any.tensor_copy(updates[ds(32 * b, N), :], pub)
        if dump_u0 and dbg_d:
            nc.sync.dma_start(dbg_d["dbg_u00"].ap(), u0)

    def slot_update(updates, qT, it, r0=0, nr=None):
        nr = NP if nr is None else nr
        # ---- GRU over stacked batches ----
        updT = small.tile([P, 2, nr], SR, tag="updT")
        slT = small.tile([P, 2, nr], SR, tag="slT")
        for c in range(2):
            tp = (r0, 0) if r0 >= 96 else None
            pt = ps.tile([P, nr], F32, tag="ps_t")
            nc.tensor.transpose(pt, updates[ds(r0, nr), ds(c * P, P)],
                                I128f[ds(r0, nr), ds(r0, nr)],
                                tile_position=tp)
            nc.any.tensor_copy(updT[:, c, :], pt)
            pt2 = ps.tile([P, nr], F32, tag="ps_t")
            nc.tensor.transpose(pt2, slots[ds(r0, nr), ds(c * P, P)],
                                I128f[ds(r0, nr), ds(r0, nr)],
                                tile_position=tp)
            nc.any.tensor_copy(slT[:, c, :], pt2)

        prz = ps.tile([nr, 512], F32, tag="ps_mm")
        first = True
        for lhsT_t, w_t in [(updT, WihT), (slT, WhhT)]:
            for dik in range(2):
                nc.tensor.matmul(prz, lhsT=lhsT_t[:, dik, :],
                                 rhs=w_t[:, dik, 0:512],
                                 start=first, stop=False)
                first = False
        nc.tensor.matmul(prz, lhsT=ones_row_b[:, :nr], rhs=bih_row[:, 0:512],
                         start=False, stop=False)
        nc.tensor.matmul(prz, lhsT=ones_row_b[:, :nr], rhs=bhh_row[:, 0:512],
                         start=False, stop=True)

        def n_half(lhsT_t, w_t, bias_row, tag):
            pg = ps3.tile([nr, D], F32, tag=tag)
            for dik in range(2):
                nc.tensor.matmul(pg, lhsT=lhsT_t[:, dik, :],
                                 rhs=w_t[:, dik, 512:768],
                                 start=dik == 0, stop=False)
            nc.tensor.matmul(pg, lhsT=ones_row_b[:, :nr],
                             rhs=bias_row[:, 512:768], start=False, stop=True)
            return pg

        pnx = n_half(updT, WihT, bih_row, "ps_s")
        pnh = n_half(slT, WhhT, bhh_row, "ps_s")

        def tt(in0, in1, op, tag="gtmp"):
            o = small.tile([nr, D], F32, tag=tag)
            nc.vector.tensor_tensor(o, in0, in1, op)
            return o

        r_g = one.tile([nr, D], F32, tag="r_g")
        nc.scalar.activation(r_g, prz[:, 0:D], AF.Sigmoid)
        z_g = one.tile([nr, D], F32, tag="z_g")
        nc.scalar.activation(z_g, prz[:, D:2 * D], AF.Sigmoid)
        t1 = tt(r_g, pnh, OP.mult)
        t2 = tt(pnx, t1, OP.add)
        n_g = one.tile([nr, D], F32, tag="n_g")
        nc.scalar.activation(n_g, t2, AF.Tanh)
        d1 = tt(slots[ds(r0, nr)], n_g, OP.subtract)
        d2 = tt(z_g, d1, OP.mult)
        nc.vector.tensor_tensor(slots[ds(r0, nr)], n_g, d2, OP.add)
        if dbg_d and it == 0:
            nc.sync.dma_start(dbg_d["dbg_q0"].ap(), qT[:])
            nc.sync.dma_start(dbg_d["dbg_upd0"].ap(), updates)
            nc.sync.dma_start(dbg_d["dbg_slotsg0"].ap(), slots)

        # ---- MLP residual ----
        stM = stats.tile([nr, 6], F32, tag="stS")
        nc.vector.bn_stats(stM, slots[ds(r0, nr)])
        mvM = stats.tile([nr, 2], F32, tag="mvS")
        nc.vector.bn_aggr(mvM, stM)
        sqM = stats.tile([nr, 1], F32, tag="sqS")
        nc.scalar.activation(sqM, mvM[:, 1:2], AF.Sqrt, bias=eps_ln[:nr])
        rcM = stats.tile([nr, 1], F32, tag="rcS")
        nc.vector.reciprocal(rcM, sqM)
        ln_h = one.tile([nr, D], F32, tag="ln_h")
        nc.vector.tensor_scalar(ln_h, slots[ds(r0, nr)], mvM[:, 0:1], rcM,
                                op0=OP.subtract, op1=OP.mult)
        lhT = small.tile([P, 2, nr], SR, tag="lhT")
        for c in range(2):
            pt = ps.tile([P, nr], F32, tag="ps_t")
            nc.tensor.transpose(pt, ln_h[:, ds(c * P, P)], I128f[:nr, :nr])
            nc.any.tensor_copy(lhT[:, c, :], pt)
        pm1 = ps.tile([nr, MLP], F32, tag="ps_mm")
        for dik in range(2):
            nc.tensor.matmul(pm1, lhsT=lhT[:, dik, :],
                             rhs=W1gT[:, dik, :],
                             start=dik == 0, stop=False)
        nc.tensor.matmul(pm1, lhsT=ones_row_b[:, :nr],
                         rhs=c1_row, start=False, stop=True)
        h1 = one.tile([nr, MLP], F32, tag="h1")
        nc.scalar.activation(h1, pm1, AF.Relu)
        h1T = one.tile([P, 4, nr], SR, tag="h1T")
        for mc in range(4):
            pt = ps.tile([P, nr], F32, tag="ps_t")
            nc.tensor.transpose(pt, h1[:, ds(mc * P, P)], I128f[:nr, :nr])
            nc.any.tensor_copy(h1T[:, mc, :], pt)
        pm2 = ps3.tile([nr, D], F32, tag="ps_s")
        for mc in range(4):
            nc.tensor.matmul(pm2, lhsT=h1T[:, mc, :],
                             rhs=W2T[:, mc, :],
                             start=mc == 0, stop=False)
        nc.tensor.matmul(pm2, lhsT=ones_row_b[:, :nr],
                         rhs=c2_row, start=False, stop=True)
        nc.vector.tensor_tensor(slots[ds(r0, nr)], slots[ds(r0, nr)], pm2,
                                OP.add)
        if dbg_d and it == 0:
            nc.sync.dma_start(dbg_d["dbg_slots0"].ap(), slots)

    # ---- pipelined schedule: iter-0 per-batch work rides phase A/B ----
    qT0 = make_q()
    upd0 = small.tile([NP, D], F32, tag="updates")
    nc.vector.memset(upd0, 0.0)
    for b in range(bl):
        phase_ab(b)
        if b > 0:
            batch_work(b - 1, qT0, upd0, dump_u0=(b == 1))
    batch_work(bl - 1, qT0, upd0, dump_u0=(bl == 1))
    if dbg_d:
        nc.sync.dma_start(dbg_d["dbg_xs"].ap(), xs[0][:, :, 0:DI])
        nc.sync.dma_start(dbg_d["dbg_kT"].ap(), kT[0][:])
    slot_update(upd0, qT0, 0)
    def tail_b(b):
        sb16 = one.tile([N, D], BF16, tag="sb16")
        nc.any.tensor_copy(sb16, slots[ds(32 * b, N), :])
        for tg in range(TC // 2):
            pa = ps1.tile([N, 256], BF16, tag="ps_c")
            for j in range(2):
                nc.tensor.transpose(pa[:, ts(j, P)],
                                    attn[b][:, tg * 2 + j, :], I128b)
            at = small.tile([N, 256], BF16, tag="at")
            nc.vector.tensor_copy(at, pa)
            so = ssout.tile([P, 2, D], F32, tag="so")
            for j in range(2):
                pss = ps3.tile([P, D], F32, tag="ps_s")
                nc.tensor.matmul(pss, lhsT=at[:, ts(j, P)], rhs=sb16,
                                 start=True, stop=True)
                if j % 2 == 0:
                    nc.scalar.activation(so[:, j, :], pss, AF.Copy)
                else:
                    nc.vector.tensor_copy(so[:, j, :], pss)
            q_eng = nc.gpsimd if tg % 2 == 0 else nc.sync
            q_eng.dma_start(
                aug_ap[b, ds(tg * 256, 256), DI:DI + D].rearrange(
                    "(j p) f -> p j f", p=P), so)

    for it in range(1, NITERS):
        qTi = make_q()
        updi = small.tile([NP, D], F32, tag="updates")
        nc.vector.memset(updi, 0.0)
        for b in range(bl):
            batch_work(b, qTi, updi)
        slot_update(updi, qTi, it)

    for b in range(bl):
        tail_b(b)
    for b in range(bl):
        nc.sync.dma_start(slots_d.ap()[b], slots[ds(32 * b, N), :])
    if dbg_d:
        nc.sync.dma_start(dbg_d["dbg_attn"].ap(), attn[0][:])


_NC_CACHE = {}


def get_nc(t_len=T, bl=BL, slot_r=True):
    key = (t_len, bl, slot_r)
    if key not in _NC_CACHE:
        _NC_CACHE[key] = build_kernel(t_len, bl, slot_r)
    return _NC_CACHE[key]


def kernel(**inputs):
    arrs = {k: np.ascontiguousarray(np.asarray(v, dtype=np.float32))
            for k, v in inputs.items()}
    nc = get_nc()
    x = arrs["x"]
    in_maps = []
    for c in range(NCORES):
        m = {name: arrs[name] for name in WEIGHT_NAMES}
        m["x"] = np.ascontiguousarray(x[c * BL:(c + 1) * BL])
        in_maps.append(m)
    res = run_bass_kernel_spmd(nc, in_maps, core_ids=list(range(NCORES)))
    aug = np.concatenate([r["aug"] for r in res.results], axis=0)
    slots = np.concatenate([r["slots_out"] for r in res.results], axis=0)
    return aug, slots
